# revision 2
# baseline (speedup 1.0000x reference)
"""CAM (channel attention module) Trainium2 kernel.

Reference computation (per sample b):
    xf = x[b].reshape(C, N)
    energy = xf @ xf.T                      # [C, C]
    att = softmax(max_row(energy) - energy) # row-wise == softmax(-energy)
    out = gamma * (att @ xf) + xf

Full shapes: x [128, 3, 16, 112, 112] f32, gamma [1] f32.
Data-parallel over batch: 16 samples per core on 8 NeuronCores.

v2: bf16 streaming pipeline.
 - input DMA casts f32->bf16 in the SWDGE (gpsimd) path: no engine
   conversion passes, SBUF holds only bf16 activations.
 - all big elementwise passes run in bf16 (DVE 2x/4x perf modes).
 - output written bf16, upconverted to f32 on host (tolerance 2e-2,
   bf16 round-trip is ~2e-3).
 - engine split: DVE does pair-products + apply chain, ScalarE does
   squares + softmax chain smalls, GpSimd issues cast-DMAs (+ optional
   offloaded ops), TensorE does the tiny partition-reduce matmuls.
"""

import sys

sys.path.insert(0, "/opt/trn_rl_repo")

import numpy as np

import concourse.bass as bass
import concourse.tile as tile
from concourse import mybir
from concourse.bass_utils import run_bass_kernel_spmd

B, C, T, H, W = 128, 3, 16, 112, 112
N = T * H * W                 # 200704
P = 128
F = N // P                    # 1568
NCORES = 8
S = B // NCORES               # 16 samples per core

FP32 = mybir.dt.float32
BF16 = mybir.dt.bfloat16
AX = mybir.AxisListType
ALU = mybir.AluOpType
ACT = mybir.ActivationFunctionType

PAIRS = [(0, 1), (0, 2), (1, 2)]

# --- tuning knobs -----------------------------------------------------------
CFG = dict(
    swdge_in=True,        # cast f32->bf16 inside the input DMA (gpsimd SWDGE)
    t1_engine="gpsimd",   # who does the 3 per-channel scalar muls
    pair_engine="vector", # who does pair-products (accum): vector|gpsimd-last
    in_bufs=3,
    out_bufs=2,
)


def _bcast_last(ap, n):
    """[p, k] -> [p, k, n] with 0-stride last dim."""
    return bass.AP(
        tensor=ap.tensor,
        offset=ap.offset,
        ap=[*ap.ap, [0, n]],
    )


def split_multi_waits(nc):
    """This container's walrus accepts only one sync-wait per instruction.
    Hoist extra waits onto single-wait NOPs on the same (in-order) queue."""
    n_split = 0
    for bb in nc.main_func.blocks:
        insts = list(bb.instructions)
        new = []
        for inst in insts:
            si = inst.sync_info
            waits = list(si.on_wait) if si is not None else []
            if len(waits) > 1:
                for i, w in enumerate(waits[:-1]):
                    nop = mybir.InstNoOp(
                        name=f"{inst.name}-wsplit{i}",
                        opcode="NoOp",
                        engine=inst.engine,
                        text_hint="wait_split",
                        bass_nofuse=True,
                        sync_info=mybir.SyncInfo(on_wait=[w], on_update=[]),
                    )
                    new.append(nop)
                    n_split += 1
                inst.sync_info = mybir.SyncInfo(
                    on_wait=[waits[-1]], on_update=list(si.on_update)
                )
            new.append(inst)
        if len(new) != len(insts):
            try:
                bb.instructions = new
            except Exception:
                del bb.instructions[:]
                bb.instructions.extend(new)
    return n_split


def build_kernel(cfg=CFG, s_per_core=S, n_free=F, split_waits=True):
    """Emit the per-core Tile program. DRAM views: [S, C, P, F]."""
    from contextlib import ExitStack

    nc = bass.Bass("TRN2", target_bir_lowering=False, debug=False)
    f = n_free

    x_d = nc.dram_tensor("x", [s_per_core, C, P, f], FP32, kind="ExternalInput")
    g_d = nc.dram_tensor("gamma", [1, 1], FP32, kind="ExternalInput")
    w2_d = nc.dram_tensor("w2c", [6, 9], FP32, kind="ExternalInput")
    i9_d = nc.dram_tensor("i9c", [1, 9], FP32, kind="ExternalInput")
    o_d = nc.dram_tensor("out", [s_per_core, C, P, f], BF16, kind="ExternalOutput")

    t1_eng = {"vector": None, "gpsimd": None, "scalar": None}

    with tile.TileContext(nc) as tc, ExitStack() as ctx:
        consts = ctx.enter_context(tc.tile_pool(name="consts", bufs=1))
        in_pool = ctx.enter_context(tc.tile_pool(name="in", bufs=cfg["in_bufs"]))
        out_pool = ctx.enter_context(tc.tile_pool(name="outp", bufs=cfg["out_bufs"]))
        sq_pool = ctx.enter_context(tc.tile_pool(name="sq", bufs=2))
        t_pool = ctx.enter_context(tc.tile_pool(name="t", bufs=2))
        small = ctx.enter_context(tc.tile_pool(name="small", bufs=4))
        psum = ctx.enter_context(tc.tile_pool(name="psum", bufs=2, space="PSUM"))

        # ---- constants ----
        ones_k = consts.tile([P, 1], FP32)          # partition-reduce rhs
        nc.vector.memset(ones_k, 1.0)
        ones_b = consts.tile([1, P], FP32)          # K=1 broadcast lhsT
        nc.vector.memset(ones_b, 1.0)
        # W2 [6, 9]: e_flat[3c+d] = partials @ W2 gather (0/1 matrix)
        w2 = consts.tile([6, 9], FP32)
        nc.sync.dma_start(out=w2, in_=w2_d.ap())
        # flat 3x3 identity
        i9 = consts.tile([1, 9], FP32)
        nc.sync.dma_start(out=i9, in_=i9_d.ap())
        gamma_sb = consts.tile([1, 1], FP32)
        nc.sync.dma_start(out=gamma_sb, in_=g_d.ap())

        xin_tiles = {}
        mb_tiles = {}
        t1_tiles = {}

        def emit_load(si):
            xin = in_pool.tile([P, C, f], BF16, tag="xin")
            src = x_d.ap()[si].rearrange("c p f -> p c f")
            if cfg["swdge_in"]:
                nc.gpsimd.dma_start(out=xin, in_=src)
            else:
                nc.sync.dma_start(out=xin, in_=src)
            xin_tiles[si] = xin

        def emit_squares(si):
            """ScalarE: 3 Square+accum passes -> partials[:, 0:3]."""
            xin = xin_tiles[si]
            partials = small.tile([P, 6], FP32, tag="partials")
            sq = sq_pool.tile([P, f], BF16, tag="sq")
            for c in range(3):
                nc.scalar.activation(
                    out=sq,
                    in_=xin[:, c, :],
                    func=ACT.Square,
                    accum_out=partials[:, c : c + 1],
                )
            return partials

        def emit_pairs(si, partials):
            """DVE (opt last one gpsimd): 3 product+accum passes."""
            xin = xin_tiles[si]
            for j, (a, b) in enumerate(PAIRS):
                tscr = t_pool.tile([P, f], BF16, tag=f"tscr_{j}")
                eng = nc.vector
                if cfg["pair_engine"] == "gpsimd-last" and j == 2:
                    eng = nc.gpsimd
                eng.scalar_tensor_tensor(
                    out=tscr,
                    in0=xin[:, a, :],
                    scalar=1.0,
                    in1=xin[:, b, :],
                    op0=ALU.mult,
                    op1=ALU.mult,
                    accum_out=partials[:, 3 + j : 4 + j],
                )

        def emit_chain(si, partials):
            # partition-reduce + gather + softmax(-e) + M broadcast
            p1t_ps = psum.tile([6, 1], FP32, tag="p1t")
            nc.tensor.matmul(out=p1t_ps, lhsT=partials, rhs=ones_k)
            p1t = small.tile([6, 1], FP32, tag="p1t_sb")
            nc.scalar.copy(p1t, p1t_ps)
            e_ps = psum.tile([1, 9], FP32, tag="e")
            nc.tensor.matmul(out=e_ps, lhsT=p1t, rhs=w2)
            e_sb = small.tile([1, 9], FP32, tag="e_sb")
            nc.scalar.copy(e_sb, e_ps)
            e3 = e_sb.rearrange("p (c d) -> p c d", d=3)
            rmin = small.tile([1, 3], FP32, tag="rmin")
            nc.vector.tensor_reduce(out=rmin, in_=e3, axis=AX.X, op=ALU.min)
            z = small.tile([1, 9], FP32, tag="z")
            nc.vector.scalar_tensor_tensor(
                out=z.rearrange("p (c d) -> p c d", d=3),
                in0=e3,
                scalar=-1.0,
                in1=_bcast_last(rmin, 3),
                op0=ALU.mult,
                op1=ALU.add,
            )
            ex = small.tile([1, 9], FP32, tag="ex")
            nc.scalar.activation(out=ex, in_=z, func=ACT.Exp)
            ex3 = ex.rearrange("p (c d) -> p c d", d=3)
            sm = small.tile([1, 3], FP32, tag="sm")
            nc.vector.tensor_reduce(out=sm, in_=ex3, axis=AX.X, op=ALU.add)
            lnsm = small.tile([1, 3], FP32, tag="lnsm")
            nc.scalar.activation(out=lnsm, in_=sm, func=ACT.Ln)
            w = small.tile([1, 9], FP32, tag="w")
            nc.vector.scalar_tensor_tensor(
                out=w.rearrange("p (c d) -> p c d", d=3),
                in0=z.rearrange("p (c d) -> p c d", d=3),
                scalar=1.0,
                in1=_bcast_last(lnsm, 3),
                op0=ALU.mult,
                op1=ALU.subtract,
            )
            att = small.tile([1, 9], FP32, tag="att")
            nc.scalar.activation(out=att, in_=w, func=ACT.Exp)
            mflat = small.tile([1, 9], FP32, tag="mflat")
            nc.vector.scalar_tensor_tensor(
                out=mflat, in0=att, scalar=gamma_sb, in1=i9, op0=ALU.mult, op1=ALU.add
            )
            mb_ps = psum.tile([P, 9], FP32, tag="mb")
            nc.tensor.matmul(out=mb_ps, lhsT=ones_b, rhs=mflat)
            mb = small.tile([P, 9], FP32, tag="mb_sb")
            nc.scalar.copy(mb, mb_ps)
            mb_tiles[si] = mb

        def emit_t1(si):
            """t1_c = x0 * mb[c,0]  (single-src scalar mul, castable engine)."""
            xin = xin_tiles[si]
            mb = mb_tiles[si]
            eng = {"vector": nc.vector, "gpsimd": nc.gpsimd, "scalar": nc.scalar}[
                cfg["t1_engine"]
            ]
            t1s = []
            for c in range(3):
                t1 = t_pool.tile([P, f], BF16, tag=f"t1_{c}")
                if cfg["t1_engine"] == "scalar":
                    nc.scalar.mul(t1, xin[:, 0, :], mb[:, 3 * c : 3 * c + 1])
                else:
                    eng.tensor_scalar_mul(t1, xin[:, 0, :], mb[:, 3 * c : 3 * c + 1])
                t1s.append(t1)
            t1_tiles[si] = t1s

        def emit_apply(si):
            xin = xin_tiles[si]
            mb = mb_tiles[si]
            t1s = t1_tiles[si]
            outt = out_pool.tile([P, C, f], BF16, tag="outt")
            for c in range(3):
                t2 = t_pool.tile([P, f], BF16, tag=f"t2_{c}")
                nc.vector.scalar_tensor_tensor(
                    out=t2,
                    in0=xin[:, 1, :],
                    scalar=mb[:, 3 * c + 1 : 3 * c + 2],
                    in1=t1s[c],
                    op0=ALU.mult,
                    op1=ALU.add,
                )
                nc.vector.scalar_tensor_tensor(
                    out=outt[:, c, :],
                    in0=xin[:, 2, :],
                    scalar=mb[:, 3 * c + 2 : 3 * c + 3],
                    in1=t2,
                    op0=ALU.mult,
                    op1=ALU.add,
                )
            nc.sync.dma_start(out=o_d.ap()[si].rearrange("c p f -> p c f"), in_=outt)
            del xin_tiles[si], mb_tiles[si], t1_tiles[si]

        # software pipeline: pairs/squares(s+1) + chain(s+1) overlap apply(s)
        emit_load(0)
        if s_per_core > 1:
            emit_load(1)
        pg = emit_squares(0)
        emit_pairs(0, pg)
        emit_chain(0, pg)
        emit_t1(0)
        for s in range(s_per_core):
            if s + 2 < s_per_core:
                emit_load(s + 2)
            if s + 1 < s_per_core:
                pg = emit_squares(s + 1)
                emit_pairs(s + 1, pg)
            emit_apply(s)
            if s + 1 < s_per_core:
                emit_chain(s + 1, pg)
                emit_t1(s + 1)

    if split_waits:
        split_multi_waits(nc)
    return nc


def const_inputs():
    w2 = np.zeros((6, 9), np.float32)
    for c in range(3):
        w2[c, 4 * c] = 1.0
    for j, (a, b) in enumerate(PAIRS):
        w2[3 + j, 3 * a + b] = 1.0
        w2[3 + j, 3 * b + a] = 1.0
    i9 = np.eye(3, dtype=np.float32).reshape(1, 9)
    return {"w2c": w2, "i9c": i9}


_NC_CACHE = {}


def _get_nc():
    key = "full"
    if key not in _NC_CACHE:
        _NC_CACHE[key] = build_kernel()
    return _NC_CACHE[key]


def kernel(x: np.ndarray, gamma: np.ndarray) -> np.ndarray:
    assert x.shape == (B, C, T, H, W) and x.dtype == np.float32
    nc = _get_nc()
    xs = np.ascontiguousarray(x).reshape(NCORES, S, C, P, F)
    g = np.asarray(gamma, dtype=np.float32).reshape(1, 1)
    cns = const_inputs()
    in_maps = [{"x": xs[i], "gamma": g, **cns} for i in range(NCORES)]
    res = run_bass_kernel_spmd(nc, in_maps, core_ids=list(range(NCORES)))
    out = np.stack(
        [np.asarray(res.results[i]["out"]).astype(np.float32) for i in range(NCORES)],
        axis=0,
    )
    return out.reshape(B, C, T, H, W)


def _install_ntff_hook():
    """The image's antenv lacks axon_hooks; synthesize it so
    run_bass_kernel_spmd(trace=True) can capture NTFF profiles."""
    import types

    try:
        from antenv.axon_hooks import get_axon_ntff_profile_hook  # noqa: F401

        return True
    except ImportError:
        pass
    try:
        import antenv

        mod = types.ModuleType("antenv.axon_hooks")
        _state = {"hook": None}

        def set_axon_ntff_profile_hook(h):
            _state["hook"] = h

        def get_axon_ntff_profile_hook():
            return _state["hook"]

        mod.set_axon_ntff_profile_hook = set_axon_ntff_profile_hook
        mod.get_axon_ntff_profile_hook = get_axon_ntff_profile_hook
        sys.modules["antenv.axon_hooks"] = mod
        antenv.axon_hooks = mod

        sys.path.insert(0, "/root/.axon_site")
        from trn_agent_boot.trn_boot import _ntff_profile_via_ctypes

        hook = _ntff_profile_via_ctypes("/opt/axon/libaxon_pjrt.so")
        if hook is None:
            return False
        set_axon_ntff_profile_hook(hook)
        return True
    except Exception as e:  # pragma: no cover
        print("ntff hook install failed:", e)
        return False


def profile_once(inputs):
    """Run with NTFF tracing; returns max per-core exec_time_ns."""
    _install_ntff_hook()
    x = np.asarray(inputs["x"])
    nc = _get_nc()
    xs = np.ascontiguousarray(x).reshape(NCORES, S, C, P, F)
    g = np.asarray(inputs["gamma"], dtype=np.float32).reshape(1, 1)
    cns = const_inputs()
    in_maps = [{"x": xs[i], "gamma": g, **cns} for i in range(NCORES)]
    res = run_bass_kernel_spmd(
        nc, in_maps, core_ids=list(range(NCORES)), trace=True
    )
    print("profile_json:", res.profile_json)
    print("exec_time_ns:", res.exec_time_ns, "mean:", res.mean_exec_time_ns)
    return res.exec_time_ns


if __name__ == "__main__":
    x = np.random.randn(B, C, T, H, W).astype(np.float32)
    gamma = np.zeros((1,), np.float32)
    y = kernel(x, gamma)
    print("ok", y.shape, float(np.abs(y - x).max()))


# revision 4
# speedup vs baseline: 4.9824x; 4.9824x over previous
"""CAM (channel attention module) Trainium2 kernel.

Reference computation (per sample b):
    xf = x[b].reshape(C, N)
    energy = xf @ xf.T                      # [C, C]
    att = softmax(max_row(energy) - energy) # row-wise == softmax(-energy)
    out = gamma * (att @ xf) + xf

Full shapes: x [128, 3, 16, 112, 112] f32, gamma [1] f32.
Data-parallel over batch: 16 samples per core on 8 NeuronCores.

v2: bf16 streaming pipeline.
 - input DMA casts f32->bf16 in the SWDGE (gpsimd) path: no engine
   conversion passes, SBUF holds only bf16 activations.
 - all big elementwise passes run in bf16 (DVE 2x/4x perf modes).
 - output written bf16, upconverted to f32 on host (tolerance 2e-2,
   bf16 round-trip is ~2e-3).
 - engine split: DVE does pair-products + apply chain, ScalarE does
   squares + softmax chain smalls, GpSimd issues cast-DMAs (+ optional
   offloaded ops), TensorE does the tiny partition-reduce matmuls.
"""

import sys

sys.path.insert(0, "/opt/trn_rl_repo")

import numpy as np

import concourse.bass as bass
import concourse.tile as tile
from concourse import mybir
from concourse.bass_utils import run_bass_kernel_spmd

B, C, T, H, W = 128, 3, 16, 112, 112
N = T * H * W                 # 200704
P = 128
F = N // P                    # 1568
NCORES = 8
S = B // NCORES               # 16 samples per core

FP32 = mybir.dt.float32
BF16 = mybir.dt.bfloat16
AX = mybir.AxisListType
ALU = mybir.AluOpType
ACT = mybir.ActivationFunctionType

PAIRS = [(0, 1), (0, 2), (1, 2)]

# --- tuning knobs -----------------------------------------------------------
CFG = dict(
    swdge_in=True,        # cast f32->bf16 inside the input DMA (gpsimd SWDGE)
    t1_engine="vector",   # who does the 3 per-channel scalar muls
    pair_engine="vector", # who does pair-products (accum): vector|gpsimd-last
    in_bufs=4,
    out_bufs=2,
)


def _bcast_last(ap, n):
    """[p, k] -> [p, k, n] with 0-stride last dim."""
    return bass.AP(
        tensor=ap.tensor,
        offset=ap.offset,
        ap=[*ap.ap, [0, n]],
    )


def split_multi_waits(nc):
    """This container's walrus accepts only one sync-wait per instruction.
    Hoist extra waits onto single-wait NOPs on the same (in-order) queue."""
    n_split = 0
    for bb in nc.main_func.blocks:
        insts = list(bb.instructions)
        new = []
        for inst in insts:
            si = inst.sync_info
            waits = list(si.on_wait) if si is not None else []
            if len(waits) > 1:
                for i, w in enumerate(waits[:-1]):
                    nop = mybir.InstNoOp(
                        name=f"{inst.name}-wsplit{i}",
                        opcode="NoOp",
                        engine=inst.engine,
                        text_hint="wait_split",
                        bass_nofuse=True,
                        sync_info=mybir.SyncInfo(on_wait=[w], on_update=[]),
                    )
                    new.append(nop)
                    n_split += 1
                inst.sync_info = mybir.SyncInfo(
                    on_wait=[waits[-1]], on_update=list(si.on_update)
                )
            new.append(inst)
        if len(new) != len(insts):
            try:
                bb.instructions = new
            except Exception:
                del bb.instructions[:]
                bb.instructions.extend(new)
    return n_split


def build_kernel(cfg=CFG, s_per_core=S, n_free=F, split_waits=True):
    """Emit the per-core Tile program. DRAM views: [S, C, P, F]."""
    from contextlib import ExitStack

    nc = bass.Bass("TRN2", target_bir_lowering=False, debug=False)
    f = n_free

    x_d = nc.dram_tensor("x", [s_per_core, C, P, f], FP32, kind="ExternalInput")
    g_d = nc.dram_tensor("gamma", [1, 1], FP32, kind="ExternalInput")
    w2_d = nc.dram_tensor("w2c", [6, 9], FP32, kind="ExternalInput")
    i9_d = nc.dram_tensor("i9c", [1, 9], FP32, kind="ExternalInput")
    o_d = nc.dram_tensor("out", [s_per_core, C, P, f], BF16, kind="ExternalOutput")

    t1_eng = {"vector": None, "gpsimd": None, "scalar": None}

    with tile.TileContext(nc) as tc, ExitStack() as ctx:
        consts = ctx.enter_context(tc.tile_pool(name="consts", bufs=1))
        in_pool = ctx.enter_context(tc.tile_pool(name="in", bufs=cfg["in_bufs"]))
        out_pool = ctx.enter_context(tc.tile_pool(name="outp", bufs=cfg["out_bufs"]))
        sq_pool = ctx.enter_context(tc.tile_pool(name="sq", bufs=2))
        t_pool = ctx.enter_context(tc.tile_pool(name="t", bufs=2))
        small = ctx.enter_context(tc.tile_pool(name="small", bufs=4))
        psum = ctx.enter_context(tc.tile_pool(name="psum", bufs=2, space="PSUM"))

        # ---- constants ----
        ones_k = consts.tile([P, 1], FP32)          # partition-reduce rhs
        nc.vector.memset(ones_k, 1.0)
        ones_b = consts.tile([1, P], FP32)          # K=1 broadcast lhsT
        nc.vector.memset(ones_b, 1.0)
        # W2 [6, 9]: e_flat[3c+d] = partials @ W2 gather (0/1 matrix)
        w2 = consts.tile([6, 9], FP32)
        nc.sync.dma_start(out=w2, in_=w2_d.ap())
        # flat 3x3 identity
        i9 = consts.tile([1, 9], FP32)
        nc.sync.dma_start(out=i9, in_=i9_d.ap())
        gamma_sb = consts.tile([1, 1], FP32)
        nc.sync.dma_start(out=gamma_sb, in_=g_d.ap())

        xin_tiles = {}
        mb_tiles = {}
        t1_tiles = {}

        def emit_load(si):
            xin = in_pool.tile([P, C, f], BF16, tag="xin")
            src = x_d.ap()[si].rearrange("c p f -> p c f")
            if cfg["swdge_in"]:
                nc.gpsimd.dma_start(out=xin, in_=src)
            else:
                nc.sync.dma_start(out=xin, in_=src)
            xin_tiles[si] = xin

        def emit_squares(si):
            """ScalarE: 3 Square+accum passes -> partials[:, 0:3]."""
            xin = xin_tiles[si]
            partials = small.tile([P, 6], FP32, tag="partials")
            sq = sq_pool.tile([P, f], BF16, tag="sq")
            for c in range(3):
                nc.scalar.activation(
                    out=sq,
                    in_=xin[:, c, :],
                    func=ACT.Square,
                    accum_out=partials[:, c : c + 1],
                )
            return partials

        def emit_pairs(si, partials):
            """DVE (opt last one gpsimd): 3 product+accum passes."""
            xin = xin_tiles[si]
            for j, (a, b) in enumerate(PAIRS):
                tscr = t_pool.tile([P, f], BF16, tag=f"tscr_{j}")
                eng = nc.vector
                if cfg["pair_engine"] == "gpsimd-last" and j == 2:
                    eng = nc.gpsimd
                eng.scalar_tensor_tensor(
                    out=tscr,
                    in0=xin[:, a, :],
                    scalar=1.0,
                    in1=xin[:, b, :],
                    op0=ALU.mult,
                    op1=ALU.mult,
                    accum_out=partials[:, 3 + j : 4 + j],
                )

        def emit_chain(si, partials):
            # partition-reduce + gather + softmax(-e) + M broadcast
            p1t_ps = psum.tile([6, 1], FP32, tag="p1t")
            nc.tensor.matmul(out=p1t_ps, lhsT=partials, rhs=ones_k)
            p1t = small.tile([6, 1], FP32, tag="p1t_sb")
            nc.scalar.copy(p1t, p1t_ps)
            e_ps = psum.tile([1, 9], FP32, tag="e")
            nc.tensor.matmul(out=e_ps, lhsT=p1t, rhs=w2)
            e_sb = small.tile([1, 9], FP32, tag="e_sb")
            nc.scalar.copy(e_sb, e_ps)
            e3 = e_sb.rearrange("p (c d) -> p c d", d=3)
            rmin = small.tile([1, 3], FP32, tag="rmin")
            nc.vector.tensor_reduce(out=rmin, in_=e3, axis=AX.X, op=ALU.min)
            z = small.tile([1, 9], FP32, tag="z")
            nc.vector.scalar_tensor_tensor(
                out=z.rearrange("p (c d) -> p c d", d=3),
                in0=e3,
                scalar=-1.0,
                in1=_bcast_last(rmin, 3),
                op0=ALU.mult,
                op1=ALU.add,
            )
            ex = small.tile([1, 9], FP32, tag="ex")
            nc.scalar.activation(out=ex, in_=z, func=ACT.Exp)
            ex3 = ex.rearrange("p (c d) -> p c d", d=3)
            sm = small.tile([1, 3], FP32, tag="sm")
            nc.vector.tensor_reduce(out=sm, in_=ex3, axis=AX.X, op=ALU.add)
            lnsm = small.tile([1, 3], FP32, tag="lnsm")
            nc.scalar.activation(out=lnsm, in_=sm, func=ACT.Ln)
            w = small.tile([1, 9], FP32, tag="w")
            nc.vector.scalar_tensor_tensor(
                out=w.rearrange("p (c d) -> p c d", d=3),
                in0=z.rearrange("p (c d) -> p c d", d=3),
                scalar=1.0,
                in1=_bcast_last(lnsm, 3),
                op0=ALU.mult,
                op1=ALU.subtract,
            )
            att = small.tile([1, 9], FP32, tag="att")
            nc.scalar.activation(out=att, in_=w, func=ACT.Exp)
            mflat = small.tile([1, 9], FP32, tag="mflat")
            nc.vector.scalar_tensor_tensor(
                out=mflat, in0=att, scalar=gamma_sb, in1=i9, op0=ALU.mult, op1=ALU.add
            )
            mb_ps = psum.tile([P, 9], FP32, tag="mb")
            nc.tensor.matmul(out=mb_ps, lhsT=ones_b, rhs=mflat)
            mb = small.tile([P, 9], FP32, tag="mb_sb")
            nc.scalar.copy(mb, mb_ps)
            mb_tiles[si] = mb

        def emit_t1(si):
            """t1_c = x0 * mb[c,0]  (single-src scalar mul, castable engine)."""
            xin = xin_tiles[si]
            mb = mb_tiles[si]
            eng = {"vector": nc.vector, "gpsimd": nc.gpsimd, "scalar": nc.scalar}[
                cfg["t1_engine"]
            ]
            t1s = []
            for c in range(3):
                t1 = t_pool.tile([P, f], BF16, tag=f"t1_{c}")
                if cfg["t1_engine"] == "scalar":
                    nc.scalar.mul(t1, xin[:, 0, :], mb[:, 3 * c : 3 * c + 1])
                else:
                    eng.tensor_scalar_mul(t1, xin[:, 0, :], mb[:, 3 * c : 3 * c + 1])
                t1s.append(t1)
            t1_tiles[si] = t1s

        def emit_apply(si):
            xin = xin_tiles[si]
            mb = mb_tiles[si]
            t1s = t1_tiles[si]
            outt = out_pool.tile([P, C, f], BF16, tag="outt")
            for c in range(3):
                t2 = t_pool.tile([P, f], BF16, tag=f"t2_{c}")
                nc.vector.scalar_tensor_tensor(
                    out=t2,
                    in0=xin[:, 1, :],
                    scalar=mb[:, 3 * c + 1 : 3 * c + 2],
                    in1=t1s[c],
                    op0=ALU.mult,
                    op1=ALU.add,
                )
                nc.vector.scalar_tensor_tensor(
                    out=outt[:, c, :],
                    in0=xin[:, 2, :],
                    scalar=mb[:, 3 * c + 2 : 3 * c + 3],
                    in1=t2,
                    op0=ALU.mult,
                    op1=ALU.add,
                )
            nc.sync.dma_start(out=o_d.ap()[si].rearrange("c p f -> p c f"), in_=outt)
            del xin_tiles[si], mb_tiles[si], t1_tiles[si]

        # software pipeline: pairs/squares(s+1) + chain(s+1) overlap apply(s)
        lookahead = cfg["in_bufs"] - 1
        for si in range(min(lookahead, s_per_core)):
            emit_load(si)
        pg = emit_squares(0)
        emit_pairs(0, pg)
        emit_chain(0, pg)
        emit_t1(0)
        for s in range(s_per_core):
            if s + lookahead < s_per_core:
                emit_load(s + lookahead)
            if s + 1 < s_per_core:
                pg = emit_squares(s + 1)
                emit_pairs(s + 1, pg)
            emit_apply(s)
            if s + 1 < s_per_core:
                emit_chain(s + 1, pg)
                emit_t1(s + 1)

    if split_waits:
        split_multi_waits(nc)
    return nc


def const_inputs():
    w2 = np.zeros((6, 9), np.float32)
    for c in range(3):
        w2[c, 4 * c] = 1.0
    for j, (a, b) in enumerate(PAIRS):
        w2[3 + j, 3 * a + b] = 1.0
        w2[3 + j, 3 * b + a] = 1.0
    i9 = np.eye(3, dtype=np.float32).reshape(1, 9)
    return {"w2c": w2, "i9c": i9}


_NC_CACHE = {}


def _get_nc():
    key = "full"
    if key not in _NC_CACHE:
        _NC_CACHE[key] = build_kernel()
    return _NC_CACHE[key]


def kernel(x: np.ndarray, gamma: np.ndarray) -> np.ndarray:
    assert x.shape == (B, C, T, H, W) and x.dtype == np.float32
    nc = _get_nc()
    xs = np.ascontiguousarray(x).reshape(NCORES, S, C, P, F)
    g = np.asarray(gamma, dtype=np.float32).reshape(1, 1)
    cns = const_inputs()
    in_maps = [{"x": xs[i], "gamma": g, **cns} for i in range(NCORES)]
    res = run_bass_kernel_spmd(nc, in_maps, core_ids=list(range(NCORES)))
    out = np.stack(
        [np.asarray(res.results[i]["out"]).astype(np.float32) for i in range(NCORES)],
        axis=0,
    )
    return out.reshape(B, C, T, H, W)


def _install_ntff_hook():
    """The image's antenv lacks axon_hooks; synthesize it so
    run_bass_kernel_spmd(trace=True) can capture NTFF profiles."""
    import types

    try:
        from antenv.axon_hooks import get_axon_ntff_profile_hook  # noqa: F401

        return True
    except ImportError:
        pass
    try:
        import antenv

        mod = types.ModuleType("antenv.axon_hooks")
        _state = {"hook": None}

        def set_axon_ntff_profile_hook(h):
            _state["hook"] = h

        def get_axon_ntff_profile_hook():
            return _state["hook"]

        mod.set_axon_ntff_profile_hook = set_axon_ntff_profile_hook
        mod.get_axon_ntff_profile_hook = get_axon_ntff_profile_hook
        sys.modules["antenv.axon_hooks"] = mod
        antenv.axon_hooks = mod

        sys.path.insert(0, "/root/.axon_site")
        from trn_agent_boot.trn_boot import _ntff_profile_via_ctypes

        hook = _ntff_profile_via_ctypes("/opt/axon/libaxon_pjrt.so")
        if hook is None:
            return False
        set_axon_ntff_profile_hook(hook)
        return True
    except Exception as e:  # pragma: no cover
        print("ntff hook install failed:", e)
        return False


def profile_once(inputs):
    """Run with NTFF tracing; returns max per-core exec_time_ns."""
    _install_ntff_hook()
    x = np.asarray(inputs["x"])
    nc = _get_nc()
    xs = np.ascontiguousarray(x).reshape(NCORES, S, C, P, F)
    g = np.asarray(inputs["gamma"], dtype=np.float32).reshape(1, 1)
    cns = const_inputs()
    in_maps = [{"x": xs[i], "gamma": g, **cns} for i in range(NCORES)]
    res = run_bass_kernel_spmd(
        nc, in_maps, core_ids=list(range(NCORES)), trace=True
    )
    print("profile_json:", res.profile_json)
    print("exec_time_ns:", res.exec_time_ns, "mean:", res.mean_exec_time_ns)
    return res.exec_time_ns


if __name__ == "__main__":
    x = np.random.randn(B, C, T, H, W).astype(np.float32)
    gamma = np.zeros((1,), np.float32)
    y = kernel(x, gamma)
    print("ok", y.shape, float(np.abs(y - x).max()))


# revision 10
# speedup vs baseline: 5.4691x; 1.0977x over previous
"""CAM (channel attention module) Trainium2 kernel.

Reference computation (per sample b):
    xf = x[b].reshape(C, N)
    energy = xf @ xf.T                      # [C, C]
    att = softmax(max_row(energy) - energy) # row-wise == softmax(-energy)
    out = gamma * (att @ xf) + xf

Full shapes: x [128, 3, 16, 112, 112] f32, gamma [1] f32.
Data-parallel over batch: 16 samples per core on 8 NeuronCores.

v5 design (per core, 16 samples, streaming):
 - input DMA casts f32->bf16 in the SWDGE (gpsimd) path; SBUF holds bf16.
 - gram on the (otherwise idle) TensorE: for each of the 6 channel pairs,
   13 accumulating [128, <=128]^T @ [128, <=128] matmuls into a
   [128, 6, 128] PSUM tile (walrus requires single-free-dim matmul APs).
   Energies = PSUM diagonals, extracted by 6 DVE STT+accum ops against an
   identity mask, partition-reduced by a ones matmul + W2 gather (tiny).
 - apply (out_c = sum_d mb[c,d] x_d, mb = I + gamma*att) as 9 bf16
   tensor_scalar muls (split DVE/ScalarE; stock TS has 2x/4x uops) and
   2 channel-fused [P, 3F] bf16 tensor_tensor adds on DVE (2x mode).
   scalar_tensor_tensor is avoided for big ops: it only has a 1x uop.
 - output written bf16 (tolerance 2e-2; bf16 round-trip ~2e-3), host
   upconverts to f32.
 - no gpsimd compute (GpSimd and DVE's 2nd port share an exclusive SBUF
   port pair; mixing them serializes both engines).
"""

import sys

sys.path.insert(0, "/opt/trn_rl_repo")

import numpy as np

import concourse.bass as bass
import concourse.tile as tile
from concourse import mybir
from concourse.bass_utils import run_bass_kernel_spmd

B, C, T, H, W = 128, 3, 16, 112, 112
N = T * H * W                 # 200704
P = 128
F = N // P                    # 1568
NCORES = 8
S = B // NCORES               # 16 samples per core

GCH = 128                     # f-chunk per gram matmul
PAIRS6 = [(0, 0), (1, 1), (2, 2), (0, 1), (0, 2), (1, 2)]

FP32 = mybir.dt.float32
BF16 = mybir.dt.bfloat16
AX = mybir.AxisListType
ALU = mybir.AluOpType
ACT = mybir.ActivationFunctionType

# --- tuning knobs -----------------------------------------------------------
CFG = dict(
    swdge_in=True,   # cast f32->bf16 inside the input DMA (gpsimd SWDGE)
    n_ts_scalar=5,   # how many of the 9 apply muls go to ScalarE (rest DVE)
    in_bufs=4,
    out_bufs=2,
    gram_bufs=2,     # PSUM double-buffering for gram tiles
)


def _bcast(ap, n, pos):
    """Insert a 0-stride dim of extent n at position pos of the ap list."""
    new = list(ap.ap)
    new.insert(pos, [0, n])
    return bass.AP(tensor=ap.tensor, offset=ap.offset, ap=new)


def split_multi_waits(nc):
    """This container's walrus accepts only one sync-wait per instruction.
    Hoist extra waits onto single-wait NOPs on the same (in-order) queue."""
    n_split = 0
    for bb in nc.main_func.blocks:
        insts = list(bb.instructions)
        new = []
        for inst in insts:
            si = inst.sync_info
            waits = list(si.on_wait) if si is not None else []
            if len(waits) > 1:
                for i, w in enumerate(waits[:-1]):
                    nop = mybir.InstNoOp(
                        name=f"{inst.name}-wsplit{i}",
                        opcode="NoOp",
                        engine=inst.engine,
                        text_hint="wait_split",
                        bass_nofuse=True,
                        sync_info=mybir.SyncInfo(on_wait=[w], on_update=[]),
                    )
                    new.append(nop)
                    n_split += 1
                inst.sync_info = mybir.SyncInfo(
                    on_wait=[waits[-1]], on_update=list(si.on_update)
                )
            new.append(inst)
        if len(new) != len(insts):
            try:
                bb.instructions = new
            except Exception:
                del bb.instructions[:]
                bb.instructions.extend(new)
    return n_split


def build_kernel(cfg=CFG, s_per_core=S, n_free=F, split_waits=True):
    """Emit the per-core Tile program. DRAM views: [S, C, P, F]."""
    from contextlib import ExitStack

    nc = bass.Bass("TRN2", target_bir_lowering=False, debug=False)
    f = n_free
    # gram chunking: 12 full 128-wide chunks + one 32-wide remainder
    chunks = []
    pos = 0
    while pos < f:
        w = min(GCH, f - pos)
        chunks.append((pos, w))
        pos += w

    x_d = nc.dram_tensor("x", [s_per_core, C, P, f], FP32, kind="ExternalInput")
    g_d = nc.dram_tensor("gamma", [1, 1], FP32, kind="ExternalInput")
    i9_d = nc.dram_tensor("i9c", [1, 9], FP32, kind="ExternalInput")
    w2_d = nc.dram_tensor("w2c", [6, 9], FP32, kind="ExternalInput")
    dg_d = nc.dram_tensor("diagm", [P, P], FP32, kind="ExternalInput")
    o_d = nc.dram_tensor("out", [s_per_core, C, P, f], BF16, kind="ExternalOutput")

    with tile.TileContext(nc) as tc, ExitStack() as ctx:
        consts = ctx.enter_context(tc.tile_pool(name="consts", bufs=1))
        in_pool = ctx.enter_context(tc.tile_pool(name="in", bufs=cfg["in_bufs"]))
        out_pool = ctx.enter_context(tc.tile_pool(name="outp", bufs=cfg["out_bufs"]))
        u_pool = ctx.enter_context(tc.tile_pool(name="u", bufs=2))
        small = ctx.enter_context(tc.tile_pool(name="small", bufs=4))
        psum = ctx.enter_context(tc.tile_pool(name="psum", bufs=1, space="PSUM"))
        gpsum = ctx.enter_context(
            tc.tile_pool(name="gpsum", bufs=cfg["gram_bufs"], space="PSUM")
        )

        # ---- constants ----
        ones_k = consts.tile([P, 1], FP32)          # partition-reduce rhs
        nc.vector.memset(ones_k, 1.0)
        ones_b = consts.tile([1, P], FP32)          # K=1 broadcast lhsT
        nc.vector.memset(ones_b, 1.0)
        i9 = consts.tile([1, 9], FP32)              # flat 3x3 identity
        nc.sync.dma_start(out=i9, in_=i9_d.ap())
        w2 = consts.tile([6, 9], FP32)              # pair -> (c,d) gather
        nc.sync.dma_start(out=w2, in_=w2_d.ap())
        diagm = consts.tile([P, P], FP32)           # 128x128 identity mask
        nc.sync.dma_start(out=diagm, in_=dg_d.ap())
        gamma_sb = consts.tile([1, 1], FP32)
        nc.sync.dma_start(out=gamma_sb, in_=g_d.ap())

        xin_tiles = {}
        gram_tiles = {}
        mb_tiles = {}

        def emit_load(si):
            xin = in_pool.tile([P, C, f], BF16, tag="xin")
            src = x_d.ap()[si].rearrange("c p f -> p c f")
            if cfg["swdge_in"]:
                nc.gpsimd.dma_start(out=xin, in_=src)
            else:
                nc.sync.dma_start(out=xin, in_=src)
            xin_tiles[si] = xin

        def emit_gram(si):
            """TensorE: M[j][f,f'] = sum_n-chunks x_c[:,f] . x_d[:,f'] per pair."""
            xin = xin_tiles[si]
            m_ps = gpsum.tile([P, 6, GCH], FP32, tag="gram")
            for k, (p0, w) in enumerate(chunks):
                for j, (a, b) in enumerate(PAIRS6):
                    nc.tensor.matmul(
                        out=m_ps[:w, j, :w],
                        lhsT=xin[:, a, p0 : p0 + w],
                        rhs=xin[:, b, p0 : p0 + w],
                        start=(k == 0),
                        stop=(k == len(chunks) - 1),
                        skip_group_check=True,
                    )
            gram_tiles[si] = m_ps

        def emit_extract(si):
            """diag sums of the 6 PSUM pair blocks -> e_sb [1, 9]."""
            m_ps = gram_tiles[si]
            dsum = small.tile([P, 6], FP32, tag="dsum")
            scr = small.tile([P, GCH], BF16, tag="scr")
            for j in range(6):
                nc.vector.scalar_tensor_tensor(
                    out=scr,
                    in0=m_ps[:, j, :],
                    scalar=1.0,
                    in1=diagm,
                    op0=ALU.mult,
                    op1=ALU.mult,
                    accum_out=dsum[:, j : j + 1],
                )
            p1t_ps = psum.tile([6, 1], FP32, tag="p1t")
            nc.tensor.matmul(out=p1t_ps, lhsT=dsum, rhs=ones_k)
            p1t = small.tile([6, 1], FP32, tag="p1t_sb")
            nc.scalar.copy(p1t, p1t_ps)
            e_ps = psum.tile([1, 9], FP32, tag="e")
            nc.tensor.matmul(out=e_ps, lhsT=p1t, rhs=w2)
            e_sb = small.tile([1, 9], FP32, tag="e_sb")
            nc.scalar.copy(e_sb, e_ps)
            del gram_tiles[si]
            return e_sb

        def emit_chain(si, e_sb):
            # softmax(-e) rows + mb = gamma*att + I, broadcast to [P, 9]
            e3 = e_sb.rearrange("p (c d) -> p c d", d=3)
            rmin = small.tile([1, 3], FP32, tag="rmin")
            nc.vector.tensor_reduce(out=rmin, in_=e3, axis=AX.X, op=ALU.min)
            z = small.tile([1, 9], FP32, tag="z")
            nc.vector.scalar_tensor_tensor(
                out=z.rearrange("p (c d) -> p c d", d=3),
                in0=e3,
                scalar=-1.0,
                in1=_bcast(rmin, 3, 2),
                op0=ALU.mult,
                op1=ALU.add,
            )
            ex = small.tile([1, 9], FP32, tag="ex")
            nc.scalar.activation(out=ex, in_=z, func=ACT.Exp)
            ex3 = ex.rearrange("p (c d) -> p c d", d=3)
            sm = small.tile([1, 3], FP32, tag="sm")
            nc.vector.tensor_reduce(out=sm, in_=ex3, axis=AX.X, op=ALU.add)
            lnsm = small.tile([1, 3], FP32, tag="lnsm")
            nc.scalar.activation(out=lnsm, in_=sm, func=ACT.Ln)
            w = small.tile([1, 9], FP32, tag="w")
            nc.vector.scalar_tensor_tensor(
                out=w.rearrange("p (c d) -> p c d", d=3),
                in0=z.rearrange("p (c d) -> p c d", d=3),
                scalar=1.0,
                in1=_bcast(lnsm, 3, 2),
                op0=ALU.mult,
                op1=ALU.subtract,
            )
            att = small.tile([1, 9], FP32, tag="att")
            nc.scalar.activation(out=att, in_=w, func=ACT.Exp)
            mflat = small.tile([1, 9], FP32, tag="mflat")
            nc.vector.scalar_tensor_tensor(
                out=mflat, in0=att, scalar=gamma_sb, in1=i9, op0=ALU.mult, op1=ALU.add
            )
            mb_ps = psum.tile([P, 9], FP32, tag="mb")
            nc.tensor.matmul(out=mb_ps, lhsT=ones_b, rhs=mflat)
            mb = small.tile([P, 9], FP32, tag="mb_sb")
            nc.scalar.copy(mb, mb_ps)
            mb_tiles[si] = mb

        def emit_apply(si):
            """U_d[:, c, :] = x_d * mb[c,d]; out = U0 + U1 + U2 (c-fused TT)."""
            xin = xin_tiles[si]
            mb = mb_tiles[si]
            u0 = u_pool.tile([P, C, f], BF16, tag="u0")
            u1 = u_pool.tile([P, C, f], BF16, tag="u1")
            u2 = u_pool.tile([P, C, f], BF16, tag="u2")
            us = [u0, u1, u2]
            # 9 scalar muls: (d, c) pairs; first n_ts_scalar on ScalarE
            order = [(d, c) for d in range(3) for c in range(3)]
            nsc = cfg["n_ts_scalar"]
            for idx, (d, c) in enumerate(order):
                dst = us[d][:, c, :]
                sc = mb[:, 3 * c + d : 3 * c + d + 1]
                if idx < nsc:
                    nc.scalar.mul(dst, xin[:, d, :], sc)
                else:
                    nc.vector.tensor_scalar_mul(dst, xin[:, d, :], sc)
            tsum = u_pool.tile([P, C, f], BF16, tag="tsum")
            nc.vector.tensor_tensor(out=tsum, in0=u0, in1=u1, op=ALU.add)
            outt = out_pool.tile([P, C, f], BF16, tag="outt")
            nc.vector.tensor_tensor(out=outt, in0=tsum, in1=u2, op=ALU.add)
            nc.sync.dma_start(out=o_d.ap()[si].rearrange("c p f -> p c f"), in_=outt)
            del xin_tiles[si], mb_tiles[si]

        # software pipeline: gram(s+1) on PE overlaps apply(s) on DVE/ScalarE
        lookahead = cfg["in_bufs"] - 1
        for si in range(min(lookahead, s_per_core)):
            emit_load(si)
        emit_gram(0)
        e0 = emit_extract(0)
        emit_chain(0, e0)
        for s in range(s_per_core):
            if s + lookahead < s_per_core:
                emit_load(s + lookahead)
            if s + 1 < s_per_core:
                emit_gram(s + 1)
            emit_apply(s)
            if s + 1 < s_per_core:
                e = emit_extract(s + 1)
                emit_chain(s + 1, e)

    if split_waits:
        split_multi_waits(nc)
    return nc


def const_inputs():
    i9 = np.eye(3, dtype=np.float32).reshape(1, 9)
    w2 = np.zeros((6, 9), np.float32)
    for j, (a, b) in enumerate(PAIRS6):
        w2[j, 3 * a + b] = 1.0
        w2[j, 3 * b + a] = 1.0
    diagm = np.eye(P, dtype=np.float32)
    return {"i9c": i9, "w2c": w2, "diagm": diagm}


_NC_CACHE = {}


def _get_nc():
    key = "full"
    if key not in _NC_CACHE:
        _NC_CACHE[key] = build_kernel()
    return _NC_CACHE[key]


def kernel(x: np.ndarray, gamma: np.ndarray) -> np.ndarray:
    assert x.shape == (B, C, T, H, W) and x.dtype == np.float32
    nc = _get_nc()
    xs = np.ascontiguousarray(x).reshape(NCORES, S, C, P, F)
    g = np.asarray(gamma, dtype=np.float32).reshape(1, 1)
    cns = const_inputs()
    in_maps = [{"x": xs[i], "gamma": g, **cns} for i in range(NCORES)]
    res = run_bass_kernel_spmd(nc, in_maps, core_ids=list(range(NCORES)))
    out = np.stack(
        [np.asarray(res.results[i]["out"]).astype(np.float32) for i in range(NCORES)],
        axis=0,
    )
    return out.reshape(B, C, T, H, W)


def _install_ntff_hook():
    """The image's antenv lacks axon_hooks; synthesize it so
    run_bass_kernel_spmd(trace=True) can capture NTFF profiles."""
    import types

    try:
        from antenv.axon_hooks import get_axon_ntff_profile_hook  # noqa: F401

        return True
    except ImportError:
        pass
    try:
        import antenv

        mod = types.ModuleType("antenv.axon_hooks")
        _state = {"hook": None}

        def set_axon_ntff_profile_hook(h):
            _state["hook"] = h

        def get_axon_ntff_profile_hook():
            return _state["hook"]

        mod.set_axon_ntff_profile_hook = set_axon_ntff_profile_hook
        mod.get_axon_ntff_profile_hook = get_axon_ntff_profile_hook
        sys.modules["antenv.axon_hooks"] = mod
        antenv.axon_hooks = mod

        sys.path.insert(0, "/root/.axon_site")
        from trn_agent_boot.trn_boot import _ntff_profile_via_ctypes

        hook = _ntff_profile_via_ctypes("/opt/axon/libaxon_pjrt.so")
        if hook is None:
            return False
        set_axon_ntff_profile_hook(hook)
        return True
    except Exception as e:  # pragma: no cover
        print("ntff hook install failed:", e)
        return False


def profile_once(inputs):
    """Run with NTFF tracing; returns max per-core exec_time_ns."""
    _install_ntff_hook()
    x = np.asarray(inputs["x"])
    nc = _get_nc()
    xs = np.ascontiguousarray(x).reshape(NCORES, S, C, P, F)
    g = np.asarray(inputs["gamma"], dtype=np.float32).reshape(1, 1)
    cns = const_inputs()
    in_maps = [{"x": xs[i], "gamma": g, **cns} for i in range(NCORES)]
    res = run_bass_kernel_spmd(
        nc, in_maps, core_ids=list(range(NCORES)), trace=True
    )
    print("profile_json:", res.profile_json)
    print("exec_time_ns:", res.exec_time_ns, "mean:", res.mean_exec_time_ns)
    return res.exec_time_ns


if __name__ == "__main__":
    x = np.random.randn(B, C, T, H, W).astype(np.float32)
    gamma = np.zeros((1,), np.float32)
    y = kernel(x, gamma)
    print("ok", y.shape, float(np.abs(y - x).max()))


# revision 12
# speedup vs baseline: 5.9786x; 1.0931x over previous
"""CAM (channel attention module) Trainium2 kernel.

Reference computation (per sample b):
    xf = x[b].reshape(C, N)
    energy = xf @ xf.T                      # [C, C]
    att = softmax(max_row(energy) - energy) # row-wise == softmax(-energy)
    out = gamma * (att @ xf) + xf

Full shapes: x [128, 3, 16, 112, 112] f32, gamma [1] f32.
Data-parallel over batch: 16 samples per core on 8 NeuronCores.

v5 design (per core, 16 samples, streaming):
 - input DMA casts f32->bf16 in the SWDGE (gpsimd) path; SBUF holds bf16.
 - gram on the (otherwise idle) TensorE: for each of the 6 channel pairs,
   13 accumulating [128, <=128]^T @ [128, <=128] matmuls into a
   [128, 6, 128] PSUM tile (walrus requires single-free-dim matmul APs).
   Energies = PSUM diagonals, extracted by 6 DVE STT+accum ops against an
   identity mask, partition-reduced by a ones matmul + W2 gather (tiny).
 - apply (out_c = sum_d mb[c,d] x_d, mb = I + gamma*att) as 9 bf16
   tensor_scalar muls (split DVE/ScalarE; stock TS has 2x/4x uops) and
   2 channel-fused [P, 3F] bf16 tensor_tensor adds on DVE (2x mode).
   scalar_tensor_tensor is avoided for big ops: it only has a 1x uop.
 - output written bf16 (tolerance 2e-2; bf16 round-trip ~2e-3), host
   upconverts to f32.
 - no gpsimd compute (GpSimd and DVE's 2nd port share an exclusive SBUF
   port pair; mixing them serializes both engines).
"""

import sys

sys.path.insert(0, "/opt/trn_rl_repo")

import numpy as np

import concourse.bass as bass
import concourse.tile as tile
from concourse import mybir
from concourse.bass_utils import run_bass_kernel_spmd

B, C, T, H, W = 128, 3, 16, 112, 112
N = T * H * W                 # 200704
P = 128
F = N // P                    # 1568
NCORES = 8
S = B // NCORES               # 16 samples per core

GCH = 128                     # f-chunk per gram matmul
PAIRS6 = [(0, 0), (1, 1), (2, 2), (0, 1), (0, 2), (1, 2)]

FP32 = mybir.dt.float32
BF16 = mybir.dt.bfloat16
AX = mybir.AxisListType
ALU = mybir.AluOpType
ACT = mybir.ActivationFunctionType

# --- tuning knobs -----------------------------------------------------------
CFG = dict(
    swdge_in=True,   # cast f32->bf16 inside the input DMA (gpsimd SWDGE)
    in_bufs=7,       # lookahead = in_bufs - 1 samples
    out_bufs=2,
    gram_bufs=2,     # PSUM double-buffering for gram tiles
    group=4,         # samples per batched softmax chain
)
# apply-mul split: ScalarE gets these (d, c) pairs, DVE the rest
MULS_SCALAR = [(0, 0), (0, 1), (0, 2), (1, 0), (1, 1)]
MULS_DVE = [(1, 2), (2, 0), (2, 1), (2, 2)]


def _bcast(ap, n, pos):
    """Insert a 0-stride dim of extent n at position pos of the ap list."""
    new = list(ap.ap)
    new.insert(pos, [0, n])
    return bass.AP(tensor=ap.tensor, offset=ap.offset, ap=new)


def split_multi_waits(nc):
    """This container's walrus accepts only one sync-wait per instruction.
    Hoist extra waits onto single-wait NOPs on the same (in-order) queue."""
    n_split = 0
    for bb in nc.main_func.blocks:
        insts = list(bb.instructions)
        new = []
        for inst in insts:
            si = inst.sync_info
            waits = list(si.on_wait) if si is not None else []
            if len(waits) > 1:
                for i, w in enumerate(waits[:-1]):
                    nop = mybir.InstNoOp(
                        name=f"{inst.name}-wsplit{i}",
                        opcode="NoOp",
                        engine=inst.engine,
                        text_hint="wait_split",
                        bass_nofuse=True,
                        sync_info=mybir.SyncInfo(on_wait=[w], on_update=[]),
                    )
                    new.append(nop)
                    n_split += 1
                inst.sync_info = mybir.SyncInfo(
                    on_wait=[waits[-1]], on_update=list(si.on_update)
                )
            new.append(inst)
        if len(new) != len(insts):
            try:
                bb.instructions = new
            except Exception:
                del bb.instructions[:]
                bb.instructions.extend(new)
    return n_split


def build_kernel(cfg=CFG, s_per_core=S, n_free=F, split_waits=True):
    """Emit the per-core Tile program. DRAM views: [S, C, P, F]."""
    from contextlib import ExitStack

    nc = bass.Bass("TRN2", target_bir_lowering=False, debug=False)
    f = n_free
    # gram chunking: 12 full 128-wide chunks + one 32-wide remainder
    chunks = []
    pos = 0
    while pos < f:
        w = min(GCH, f - pos)
        chunks.append((pos, w))
        pos += w

    x_d = nc.dram_tensor("x", [s_per_core, C, P, f], FP32, kind="ExternalInput")
    g_d = nc.dram_tensor("gamma", [1, 1], FP32, kind="ExternalInput")
    i9_d = nc.dram_tensor("i9c", [1, 9], FP32, kind="ExternalInput")
    w2_d = nc.dram_tensor("w2c", [6, 9], FP32, kind="ExternalInput")
    dg_d = nc.dram_tensor("diagm", [P, P], FP32, kind="ExternalInput")
    w2g_d = nc.dram_tensor(
        "w2g", [6 * cfg["group"], 9 * cfg["group"]], FP32, kind="ExternalInput"
    )
    i9g_d = nc.dram_tensor("i9g", [1, 9 * cfg["group"]], FP32, kind="ExternalInput")
    o_d = nc.dram_tensor("out", [s_per_core, C, P, f], BF16, kind="ExternalOutput")

    with tile.TileContext(nc) as tc, ExitStack() as ctx:
        consts = ctx.enter_context(tc.tile_pool(name="consts", bufs=1))
        in_pool = ctx.enter_context(tc.tile_pool(name="in", bufs=cfg["in_bufs"]))
        out_pool = ctx.enter_context(tc.tile_pool(name="outp", bufs=cfg["out_bufs"]))
        u_pool = ctx.enter_context(tc.tile_pool(name="u", bufs=2))
        small = ctx.enter_context(tc.tile_pool(name="small", bufs=4))
        psum = ctx.enter_context(tc.tile_pool(name="psum", bufs=1, space="PSUM"))
        gpsum = ctx.enter_context(
            tc.tile_pool(name="gpsum", bufs=cfg["gram_bufs"], space="PSUM")
        )

        # ---- constants ----
        ones_k = consts.tile([P, 1], FP32)          # partition-reduce rhs
        nc.vector.memset(ones_k, 1.0)
        ones_b = consts.tile([1, P], FP32)          # K=1 broadcast lhsT
        nc.vector.memset(ones_b, 1.0)
        i9 = consts.tile([1, 9], FP32)              # flat 3x3 identity
        nc.sync.dma_start(out=i9, in_=i9_d.ap())
        w2 = consts.tile([6, 9], FP32)              # pair -> (c,d) gather
        nc.sync.dma_start(out=w2, in_=w2_d.ap())
        w2g = consts.tile([6 * CFG["group"], 9 * CFG["group"]], FP32)
        nc.sync.dma_start(out=w2g, in_=w2g_d.ap())
        i9g = consts.tile([1, 9 * CFG["group"]], FP32)
        nc.sync.dma_start(out=i9g, in_=i9g_d.ap())
        diagm = consts.tile([P, P], FP32)           # 128x128 identity mask
        nc.sync.dma_start(out=diagm, in_=dg_d.ap())
        gamma_sb = consts.tile([1, 1], FP32)
        nc.sync.dma_start(out=gamma_sb, in_=g_d.ap())

        xin_tiles = {}
        gram_tiles = {}
        mb_tiles = {}

        def emit_load(si):
            xin = in_pool.tile([P, C, f], BF16, tag="xin")
            src = x_d.ap()[si].rearrange("c p f -> p c f")
            if cfg["swdge_in"]:
                nc.gpsimd.dma_start(out=xin, in_=src)
            else:
                nc.sync.dma_start(out=xin, in_=src)
            xin_tiles[si] = xin

        def emit_gram(si):
            """TensorE: M[j][f,f'] = sum_n-chunks x_c[:,f] . x_d[:,f'] per pair."""
            xin = xin_tiles[si]
            m_ps = gpsum.tile([P, 6, GCH], FP32, tag="gram")
            for k, (p0, w) in enumerate(chunks):
                for j, (a, b) in enumerate(PAIRS6):
                    nc.tensor.matmul(
                        out=m_ps[:w, j, :w],
                        lhsT=xin[:, a, p0 : p0 + w],
                        rhs=xin[:, b, p0 : p0 + w],
                        start=(k == 0),
                        stop=(k == len(chunks) - 1),
                        skip_group_check=True,
                    )
            gram_tiles[si] = m_ps

        def emit_extract(si, dsum, slot):
            """diag sums of the 6 PSUM pair blocks -> dsum[:, 6*slot:6*slot+6]."""
            m_ps = gram_tiles[si]
            scr = small.tile([P, GCH], BF16, tag="scr")
            for j in range(6):
                nc.vector.scalar_tensor_tensor(
                    out=scr,
                    in0=m_ps[:, j, :],
                    scalar=1.0,
                    in1=diagm,
                    op0=ALU.mult,
                    op1=ALU.mult,
                    accum_out=dsum[:, 6 * slot + j : 6 * slot + j + 1],
                )
            del gram_tiles[si]

        GRP = cfg["group"]

        def emit_chain_group(g, dsum):
            """batched softmax chain for GRP samples -> mb [P, 9*GRP]."""
            n9 = 9 * GRP
            p1t_ps = psum.tile([6 * GRP, 1], FP32, tag="p1t")
            nc.tensor.matmul(out=p1t_ps, lhsT=dsum, rhs=ones_k)
            p1t = small.tile([6 * GRP, 1], FP32, tag="p1t_sb")
            nc.scalar.copy(p1t, p1t_ps)
            e_ps = psum.tile([1, n9], FP32, tag="e")
            nc.tensor.matmul(out=e_ps, lhsT=p1t, rhs=w2g)
            e_sb = small.tile([1, n9], FP32, tag="e_sb")
            nc.scalar.copy(e_sb, e_ps)
            e3 = e_sb.rearrange("p (sc d) -> p sc d", d=3)
            rmin = small.tile([1, 3 * GRP], FP32, tag="rmin")
            nc.vector.tensor_reduce(out=rmin, in_=e3, axis=AX.X, op=ALU.min)
            z = small.tile([1, n9], FP32, tag="z")
            nc.vector.scalar_tensor_tensor(
                out=z.rearrange("p (sc d) -> p sc d", d=3),
                in0=e3,
                scalar=-1.0,
                in1=_bcast(rmin, 3, 2),
                op0=ALU.mult,
                op1=ALU.add,
            )
            ex = small.tile([1, n9], FP32, tag="ex")
            nc.scalar.activation(out=ex, in_=z, func=ACT.Exp)
            ex3 = ex.rearrange("p (sc d) -> p sc d", d=3)
            sm = small.tile([1, 3 * GRP], FP32, tag="sm")
            nc.vector.tensor_reduce(out=sm, in_=ex3, axis=AX.X, op=ALU.add)
            lnsm = small.tile([1, 3 * GRP], FP32, tag="lnsm")
            nc.scalar.activation(out=lnsm, in_=sm, func=ACT.Ln)
            w = small.tile([1, n9], FP32, tag="w")
            nc.vector.scalar_tensor_tensor(
                out=w.rearrange("p (sc d) -> p sc d", d=3),
                in0=z.rearrange("p (sc d) -> p sc d", d=3),
                scalar=1.0,
                in1=_bcast(lnsm, 3, 2),
                op0=ALU.mult,
                op1=ALU.subtract,
            )
            att = small.tile([1, n9], FP32, tag="att")
            nc.scalar.activation(out=att, in_=w, func=ACT.Exp)
            mflat = small.tile([1, n9], FP32, tag="mflat")
            nc.vector.scalar_tensor_tensor(
                out=mflat, in0=att, scalar=gamma_sb, in1=i9g, op0=ALU.mult, op1=ALU.add
            )
            mb_ps = psum.tile([P, n9], FP32, tag="mb")
            nc.tensor.matmul(out=mb_ps, lhsT=ones_b, rhs=mflat)
            mb = small.tile([P, n9], FP32, tag="mb_sb")
            nc.scalar.copy(mb, mb_ps)
            mb_tiles[g] = mb

        u_tiles = {}

        def emit_muls(si):
            """U_d[:, c, :] = x_d * mb[c,d] (9 scalar muls, split engines)."""
            xin = xin_tiles[si]
            mb = mb_tiles[si // GRP]
            off = 9 * (si % GRP)
            u0 = u_pool.tile([P, C, f], BF16, tag="u0")
            u1 = u_pool.tile([P, C, f], BF16, tag="u1")
            u2 = u_pool.tile([P, C, f], BF16, tag="u2")
            us = [u0, u1, u2]
            for d, c in MULS_SCALAR:
                nc.scalar.mul(
                    us[d][:, c, :], xin[:, d, :],
                    mb[:, off + 3 * c + d : off + 3 * c + d + 1],
                )
            for d, c in MULS_DVE:
                nc.vector.tensor_scalar_mul(
                    us[d][:, c, :], xin[:, d, :],
                    mb[:, off + 3 * c + d : off + 3 * c + d + 1],
                )
            u_tiles[si] = us
            del xin_tiles[si]

        def emit_apply_tt(si):
            """out = U0 + U1 + U2 (c-fused TT adds) + store."""
            u0, u1, u2 = u_tiles[si]
            tsum = u_pool.tile([P, C, f], BF16, tag="tsum")
            nc.vector.tensor_tensor(out=tsum, in0=u0, in1=u1, op=ALU.add)
            outt = out_pool.tile([P, C, f], BF16, tag="outt")
            nc.vector.tensor_tensor(out=outt, in0=tsum, in1=u2, op=ALU.add)
            nc.sync.dma_start(out=o_d.ap()[si].rearrange("c p f -> p c f"), in_=outt)
            del u_tiles[si]

        # ---- software pipeline ----
        # U muls run one sample ahead; chains batched per GRP samples and
        # computed one group ahead; gram(s+GRP) on PE overlaps apply(s).
        lookahead = cfg["in_bufs"] - 1
        for si in range(min(lookahead, s_per_core)):
            emit_load(si)
        dsum = small.tile([P, 6 * GRP], FP32, tag="dsum", name="dsum_p")
        for k in range(min(GRP, s_per_core)):
            emit_gram(k)
            emit_extract(k, dsum, k)
        emit_chain_group(0, dsum)
        emit_muls(0)
        for s in range(s_per_core):
            if s + lookahead < s_per_core:
                emit_load(s + lookahead)
            s2 = s + GRP
            if s2 < s_per_core:
                if s2 % GRP == 0:
                    dsum = small.tile([P, 6 * GRP], FP32, tag="dsum", name="dsum_l")
                emit_gram(s2)
            emit_apply_tt(s)
            if s2 < s_per_core:
                emit_extract(s2, dsum, s2 % GRP)
                if s2 % GRP == GRP - 1:
                    emit_chain_group(s2 // GRP, dsum)
            if s + 1 < s_per_core:
                emit_muls(s + 1)

    if split_waits:
        split_multi_waits(nc)
    return nc


def const_inputs():
    i9 = np.eye(3, dtype=np.float32).reshape(1, 9)
    w2 = np.zeros((6, 9), np.float32)
    for j, (a, b) in enumerate(PAIRS6):
        w2[j, 3 * a + b] = 1.0
        w2[j, 3 * b + a] = 1.0
    diagm = np.eye(P, dtype=np.float32)
    g = CFG["group"]
    w2g = np.kron(np.eye(g, dtype=np.float32), w2)
    i9g = np.tile(i9, (1, g))
    return {"i9c": i9, "w2c": w2, "diagm": diagm, "w2g": w2g, "i9g": i9g}


_NC_CACHE = {}


def _get_nc():
    key = "full"
    if key not in _NC_CACHE:
        _NC_CACHE[key] = build_kernel()
    return _NC_CACHE[key]


def kernel(x: np.ndarray, gamma: np.ndarray) -> np.ndarray:
    assert x.shape == (B, C, T, H, W) and x.dtype == np.float32
    nc = _get_nc()
    xs = np.ascontiguousarray(x).reshape(NCORES, S, C, P, F)
    g = np.asarray(gamma, dtype=np.float32).reshape(1, 1)
    cns = const_inputs()
    in_maps = [{"x": xs[i], "gamma": g, **cns} for i in range(NCORES)]
    res = run_bass_kernel_spmd(nc, in_maps, core_ids=list(range(NCORES)))
    out = np.stack(
        [np.asarray(res.results[i]["out"]).astype(np.float32) for i in range(NCORES)],
        axis=0,
    )
    return out.reshape(B, C, T, H, W)


def _install_ntff_hook():
    """The image's antenv lacks axon_hooks; synthesize it so
    run_bass_kernel_spmd(trace=True) can capture NTFF profiles."""
    import types

    try:
        from antenv.axon_hooks import get_axon_ntff_profile_hook  # noqa: F401

        return True
    except ImportError:
        pass
    try:
        import antenv

        mod = types.ModuleType("antenv.axon_hooks")
        _state = {"hook": None}

        def set_axon_ntff_profile_hook(h):
            _state["hook"] = h

        def get_axon_ntff_profile_hook():
            return _state["hook"]

        mod.set_axon_ntff_profile_hook = set_axon_ntff_profile_hook
        mod.get_axon_ntff_profile_hook = get_axon_ntff_profile_hook
        sys.modules["antenv.axon_hooks"] = mod
        antenv.axon_hooks = mod

        sys.path.insert(0, "/root/.axon_site")
        from trn_agent_boot.trn_boot import _ntff_profile_via_ctypes

        hook = _ntff_profile_via_ctypes("/opt/axon/libaxon_pjrt.so")
        if hook is None:
            return False
        set_axon_ntff_profile_hook(hook)
        return True
    except Exception as e:  # pragma: no cover
        print("ntff hook install failed:", e)
        return False


def profile_once(inputs):
    """Run with NTFF tracing; returns max per-core exec_time_ns."""
    _install_ntff_hook()
    x = np.asarray(inputs["x"])
    nc = _get_nc()
    xs = np.ascontiguousarray(x).reshape(NCORES, S, C, P, F)
    g = np.asarray(inputs["gamma"], dtype=np.float32).reshape(1, 1)
    cns = const_inputs()
    in_maps = [{"x": xs[i], "gamma": g, **cns} for i in range(NCORES)]
    res = run_bass_kernel_spmd(
        nc, in_maps, core_ids=list(range(NCORES)), trace=True
    )
    print("profile_json:", res.profile_json)
    print("exec_time_ns:", res.exec_time_ns, "mean:", res.mean_exec_time_ns)
    return res.exec_time_ns


if __name__ == "__main__":
    x = np.random.randn(B, C, T, H, W).astype(np.float32)
    gamma = np.zeros((1,), np.float32)
    y = kernel(x, gamma)
    print("ok", y.shape, float(np.abs(y - x).max()))


# revision 13
# speedup vs baseline: 6.8909x; 1.1526x over previous
"""CAM (channel attention module) Trainium2 kernel.

Reference computation (per sample b):
    xf = x[b].reshape(C, N)
    energy = xf @ xf.T                      # [C, C]
    att = softmax(max_row(energy) - energy) # row-wise == softmax(-energy)
    out = gamma * (att @ xf) + xf

Full shapes: x [128, 3, 16, 112, 112] f32, gamma [1] f32.
Data-parallel over batch: 16 samples per core on 8 NeuronCores.

v5 design (per core, 16 samples, streaming):
 - input DMA casts f32->bf16 in the SWDGE (gpsimd) path; SBUF holds bf16.
 - gram on the (otherwise idle) TensorE: for each of the 6 channel pairs,
   13 accumulating [128, <=128]^T @ [128, <=128] matmuls into a
   [128, 6, 128] PSUM tile (walrus requires single-free-dim matmul APs).
   Energies = PSUM diagonals, extracted by 6 DVE STT+accum ops against an
   identity mask, partition-reduced by a ones matmul + W2 gather (tiny).
 - apply (out_c = sum_d mb[c,d] x_d, mb = I + gamma*att) as 9 bf16
   tensor_scalar muls (split DVE/ScalarE; stock TS has 2x/4x uops) and
   2 channel-fused [P, 3F] bf16 tensor_tensor adds on DVE (2x mode).
   scalar_tensor_tensor is avoided for big ops: it only has a 1x uop.
 - output written bf16 (tolerance 2e-2; bf16 round-trip ~2e-3), host
   upconverts to f32.
 - no gpsimd compute (GpSimd and DVE's 2nd port share an exclusive SBUF
   port pair; mixing them serializes both engines).
"""

import sys

sys.path.insert(0, "/opt/trn_rl_repo")

import numpy as np

import concourse.bass as bass
import concourse.tile as tile
from concourse import mybir
from concourse.bass_utils import run_bass_kernel_spmd

B, C, T, H, W = 128, 3, 16, 112, 112
N = T * H * W                 # 200704
P = 128
F = N // P                    # 1568
NCORES = 8
S = B // NCORES               # 16 samples per core

GCH = 128                     # f-chunk per gram matmul
PAIRS6 = [(0, 0), (1, 1), (2, 2), (0, 1), (0, 2), (1, 2)]

FP32 = mybir.dt.float32
BF16 = mybir.dt.bfloat16
AX = mybir.AxisListType
ALU = mybir.AluOpType
ACT = mybir.ActivationFunctionType

# --- tuning knobs -----------------------------------------------------------
CFG = dict(
    swdge_in=True,   # cast f32->bf16 inside the input DMA (gpsimd SWDGE)
    ahead=6,         # gram/extract run this many samples ahead of apply
    in_bufs=9,       # lookahead = in_bufs - 1 samples
    out_bufs=2,
    gram_bufs=2,     # PSUM double-buffering for gram tiles
    group=4,         # samples per batched softmax chain
)
# apply-mul split: ScalarE gets these (d, c) pairs, DVE the rest
MULS_SCALAR = [(0, 0), (0, 1), (0, 2), (1, 0), (1, 1)]
MULS_DVE = [(1, 2), (2, 0), (2, 1), (2, 2)]


def _bcast(ap, n, pos):
    """Insert a 0-stride dim of extent n at position pos of the ap list."""
    new = list(ap.ap)
    new.insert(pos, [0, n])
    return bass.AP(tensor=ap.tensor, offset=ap.offset, ap=new)


def split_multi_waits(nc):
    """This container's walrus accepts only one sync-wait per instruction.
    Hoist extra waits onto single-wait NOPs on the same (in-order) queue."""
    n_split = 0
    for bb in nc.main_func.blocks:
        insts = list(bb.instructions)
        new = []
        for inst in insts:
            si = inst.sync_info
            waits = list(si.on_wait) if si is not None else []
            if len(waits) > 1:
                for i, w in enumerate(waits[:-1]):
                    nop = mybir.InstNoOp(
                        name=f"{inst.name}-wsplit{i}",
                        opcode="NoOp",
                        engine=inst.engine,
                        text_hint="wait_split",
                        bass_nofuse=True,
                        sync_info=mybir.SyncInfo(on_wait=[w], on_update=[]),
                    )
                    new.append(nop)
                    n_split += 1
                inst.sync_info = mybir.SyncInfo(
                    on_wait=[waits[-1]], on_update=list(si.on_update)
                )
            new.append(inst)
        if len(new) != len(insts):
            try:
                bb.instructions = new
            except Exception:
                del bb.instructions[:]
                bb.instructions.extend(new)
    return n_split


def build_kernel(cfg=CFG, s_per_core=S, n_free=F, split_waits=True):
    """Emit the per-core Tile program. DRAM views: [S, C, P, F]."""
    from contextlib import ExitStack

    nc = bass.Bass("TRN2", target_bir_lowering=False, debug=False)
    f = n_free
    # gram chunking: 12 full 128-wide chunks + one 32-wide remainder
    chunks = []
    pos = 0
    while pos < f:
        w = min(GCH, f - pos)
        chunks.append((pos, w))
        pos += w

    x_d = nc.dram_tensor("x", [s_per_core, C, P, f], FP32, kind="ExternalInput")
    g_d = nc.dram_tensor("gamma", [1, 1], FP32, kind="ExternalInput")
    i9_d = nc.dram_tensor("i9c", [1, 9], FP32, kind="ExternalInput")
    w2_d = nc.dram_tensor("w2c", [6, 9], FP32, kind="ExternalInput")
    dg_d = nc.dram_tensor("diagm", [P, P], FP32, kind="ExternalInput")
    w2g_d = nc.dram_tensor(
        "w2g", [6 * cfg["group"], 9 * cfg["group"]], FP32, kind="ExternalInput"
    )
    i9g_d = nc.dram_tensor("i9g", [1, 9 * cfg["group"]], FP32, kind="ExternalInput")
    o_d = nc.dram_tensor("out", [s_per_core, C, P, f], BF16, kind="ExternalOutput")

    with tile.TileContext(nc) as tc, ExitStack() as ctx:
        consts = ctx.enter_context(tc.tile_pool(name="consts", bufs=1))
        in_pool = ctx.enter_context(tc.tile_pool(name="in", bufs=cfg["in_bufs"]))
        out_pool = ctx.enter_context(tc.tile_pool(name="outp", bufs=cfg["out_bufs"]))
        u_pool = ctx.enter_context(tc.tile_pool(name="u", bufs=2))
        small = ctx.enter_context(tc.tile_pool(name="small", bufs=4))
        psum = ctx.enter_context(tc.tile_pool(name="psum", bufs=1, space="PSUM"))
        gpsum = ctx.enter_context(
            tc.tile_pool(name="gpsum", bufs=cfg["gram_bufs"], space="PSUM")
        )

        # ---- constants ----
        ones_k = consts.tile([P, 1], FP32)          # partition-reduce rhs
        nc.vector.memset(ones_k, 1.0)
        ones_b = consts.tile([1, P], FP32)          # K=1 broadcast lhsT
        nc.vector.memset(ones_b, 1.0)
        i9 = consts.tile([1, 9], FP32)              # flat 3x3 identity
        nc.sync.dma_start(out=i9, in_=i9_d.ap())
        w2 = consts.tile([6, 9], FP32)              # pair -> (c,d) gather
        nc.sync.dma_start(out=w2, in_=w2_d.ap())
        w2g = consts.tile([6 * CFG["group"], 9 * CFG["group"]], FP32)
        nc.sync.dma_start(out=w2g, in_=w2g_d.ap())
        i9g = consts.tile([1, 9 * CFG["group"]], FP32)
        nc.sync.dma_start(out=i9g, in_=i9g_d.ap())
        diagm = consts.tile([P, P], FP32)           # 128x128 identity mask
        nc.sync.dma_start(out=diagm, in_=dg_d.ap())
        gamma_sb = consts.tile([1, 1], FP32)
        nc.sync.dma_start(out=gamma_sb, in_=g_d.ap())

        xin_tiles = {}
        gram_tiles = {}
        mb_tiles = {}

        def emit_load(si):
            xin = in_pool.tile([P, C, f], BF16, tag="xin")
            src = x_d.ap()[si].rearrange("c p f -> p c f")
            if cfg["swdge_in"]:
                nc.gpsimd.dma_start(out=xin, in_=src)
            else:
                nc.sync.dma_start(out=xin, in_=src)
            xin_tiles[si] = xin

        def emit_gram(si):
            """TensorE: M[j][f,f'] = sum_n-chunks x_c[:,f] . x_d[:,f'] per pair."""
            xin = xin_tiles[si]
            m_ps = gpsum.tile([P, 6, GCH], FP32, tag="gram")
            for k, (p0, w) in enumerate(chunks):
                for j, (a, b) in enumerate(PAIRS6):
                    nc.tensor.matmul(
                        out=m_ps[:w, j, :w],
                        lhsT=xin[:, a, p0 : p0 + w],
                        rhs=xin[:, b, p0 : p0 + w],
                        start=(k == 0),
                        stop=(k == len(chunks) - 1),
                        skip_group_check=True,
                    )
            gram_tiles[si] = m_ps

        def emit_extract(si, dsum, slot):
            """diag sums of the 6 PSUM pair blocks -> dsum[:, 6*slot:6*slot+6]."""
            m_ps = gram_tiles[si]
            scr = small.tile([P, GCH], BF16, tag="scr")
            for j in range(6):
                nc.vector.scalar_tensor_tensor(
                    out=scr,
                    in0=m_ps[:, j, :],
                    scalar=1.0,
                    in1=diagm,
                    op0=ALU.mult,
                    op1=ALU.mult,
                    accum_out=dsum[:, 6 * slot + j : 6 * slot + j + 1],
                )
            del gram_tiles[si]

        GRP = cfg["group"]

        def emit_chain_group(g, dsum):
            """batched softmax chain for GRP samples -> mb [P, 9*GRP]."""
            n9 = 9 * GRP
            p1t_ps = psum.tile([6 * GRP, 1], FP32, tag="p1t")
            nc.tensor.matmul(out=p1t_ps, lhsT=dsum, rhs=ones_k)
            p1t = small.tile([6 * GRP, 1], FP32, tag="p1t_sb")
            nc.scalar.copy(p1t, p1t_ps)
            e_ps = psum.tile([1, n9], FP32, tag="e")
            nc.tensor.matmul(out=e_ps, lhsT=p1t, rhs=w2g)
            e_sb = small.tile([1, n9], FP32, tag="e_sb")
            nc.scalar.copy(e_sb, e_ps)
            e3 = e_sb.rearrange("p (sc d) -> p sc d", d=3)
            rmin = small.tile([1, 3 * GRP], FP32, tag="rmin")
            nc.vector.tensor_reduce(out=rmin, in_=e3, axis=AX.X, op=ALU.min)
            z = small.tile([1, n9], FP32, tag="z")
            nc.vector.scalar_tensor_tensor(
                out=z.rearrange("p (sc d) -> p sc d", d=3),
                in0=e3,
                scalar=-1.0,
                in1=_bcast(rmin, 3, 2),
                op0=ALU.mult,
                op1=ALU.add,
            )
            ex = small.tile([1, n9], FP32, tag="ex")
            nc.scalar.activation(out=ex, in_=z, func=ACT.Exp)
            ex3 = ex.rearrange("p (sc d) -> p sc d", d=3)
            sm = small.tile([1, 3 * GRP], FP32, tag="sm")
            nc.vector.tensor_reduce(out=sm, in_=ex3, axis=AX.X, op=ALU.add)
            lnsm = small.tile([1, 3 * GRP], FP32, tag="lnsm")
            nc.scalar.activation(out=lnsm, in_=sm, func=ACT.Ln)
            w = small.tile([1, n9], FP32, tag="w")
            nc.vector.scalar_tensor_tensor(
                out=w.rearrange("p (sc d) -> p sc d", d=3),
                in0=z.rearrange("p (sc d) -> p sc d", d=3),
                scalar=1.0,
                in1=_bcast(lnsm, 3, 2),
                op0=ALU.mult,
                op1=ALU.subtract,
            )
            att = small.tile([1, n9], FP32, tag="att")
            nc.scalar.activation(out=att, in_=w, func=ACT.Exp)
            mflat = small.tile([1, n9], FP32, tag="mflat")
            nc.vector.scalar_tensor_tensor(
                out=mflat, in0=att, scalar=gamma_sb, in1=i9g, op0=ALU.mult, op1=ALU.add
            )
            mb_ps = psum.tile([P, n9], FP32, tag="mb")
            nc.tensor.matmul(out=mb_ps, lhsT=ones_b, rhs=mflat)
            mb = small.tile([P, n9], FP32, tag="mb_sb")
            nc.scalar.copy(mb, mb_ps)
            mb_tiles[g] = mb

        u_tiles = {}

        def emit_muls(si):
            """U_d[:, c, :] = x_d * mb[c,d] (9 scalar muls, split engines)."""
            xin = xin_tiles[si]
            mb = mb_tiles[si // GRP]
            off = 9 * (si % GRP)
            u0 = u_pool.tile([P, C, f], BF16, tag="u0")
            u1 = u_pool.tile([P, C, f], BF16, tag="u1")
            u2 = u_pool.tile([P, C, f], BF16, tag="u2")
            us = [u0, u1, u2]
            for d, c in MULS_SCALAR:
                nc.scalar.mul(
                    us[d][:, c, :], xin[:, d, :],
                    mb[:, off + 3 * c + d : off + 3 * c + d + 1],
                )
            for d, c in MULS_DVE:
                nc.vector.tensor_scalar_mul(
                    us[d][:, c, :], xin[:, d, :],
                    mb[:, off + 3 * c + d : off + 3 * c + d + 1],
                )
            u_tiles[si] = us
            del xin_tiles[si]

        def emit_apply_tt(si):
            """out = U0 + U1 + U2 (c-fused TT adds) + store."""
            u0, u1, u2 = u_tiles[si]
            tsum = u_pool.tile([P, C, f], BF16, tag="tsum")
            nc.vector.tensor_tensor(out=tsum, in0=u0, in1=u1, op=ALU.add)
            outt = out_pool.tile([P, C, f], BF16, tag="outt")
            nc.vector.tensor_tensor(out=outt, in0=tsum, in1=u2, op=ALU.add)
            nc.sync.dma_start(out=o_d.ap()[si].rearrange("c p f -> p c f"), in_=outt)
            del u_tiles[si]

        # ---- software pipeline ----
        # U muls run one sample ahead of the TT adds; gram/extract run
        # AHEAD samples ahead so the batched chain (one per GRP samples)
        # lands ~2 iterations before its group's muls need mb.
        AHEAD = cfg["ahead"]
        lookahead = cfg["in_bufs"] - 1
        dsums = {}

        def group_dsum(g):
            if g not in dsums:
                dsums[g] = small.tile([P, 6 * GRP], FP32, tag="dsum", name=f"ds{g}")
            return dsums[g]

        for si in range(min(lookahead, s_per_core)):
            emit_load(si)
        for si in range(min(AHEAD, s_per_core)):
            emit_gram(si)
            emit_extract(si, group_dsum(si // GRP), si % GRP)
            if si % GRP == GRP - 1:
                emit_chain_group(si // GRP, dsums[si // GRP])
        emit_muls(0)
        for s in range(s_per_core):
            if s + lookahead < s_per_core:
                emit_load(s + lookahead)
            s2 = s + AHEAD
            if s2 < s_per_core:
                emit_gram(s2)
            emit_apply_tt(s)
            if s2 < s_per_core:
                emit_extract(s2, group_dsum(s2 // GRP), s2 % GRP)
            if s + 1 < s_per_core:
                emit_muls(s + 1)
            if s2 < s_per_core and s2 % GRP == GRP - 1:
                emit_chain_group(s2 // GRP, dsums[s2 // GRP])

    if split_waits:
        split_multi_waits(nc)
    return nc


def const_inputs():
    i9 = np.eye(3, dtype=np.float32).reshape(1, 9)
    w2 = np.zeros((6, 9), np.float32)
    for j, (a, b) in enumerate(PAIRS6):
        w2[j, 3 * a + b] = 1.0
        w2[j, 3 * b + a] = 1.0
    diagm = np.eye(P, dtype=np.float32)
    g = CFG["group"]
    w2g = np.kron(np.eye(g, dtype=np.float32), w2)
    i9g = np.tile(i9, (1, g))
    return {"i9c": i9, "w2c": w2, "diagm": diagm, "w2g": w2g, "i9g": i9g}


_NC_CACHE = {}


def _get_nc():
    key = "full"
    if key not in _NC_CACHE:
        _NC_CACHE[key] = build_kernel()
    return _NC_CACHE[key]


def kernel(x: np.ndarray, gamma: np.ndarray) -> np.ndarray:
    assert x.shape == (B, C, T, H, W) and x.dtype == np.float32
    nc = _get_nc()
    xs = np.ascontiguousarray(x).reshape(NCORES, S, C, P, F)
    g = np.asarray(gamma, dtype=np.float32).reshape(1, 1)
    cns = const_inputs()
    in_maps = [{"x": xs[i], "gamma": g, **cns} for i in range(NCORES)]
    res = run_bass_kernel_spmd(nc, in_maps, core_ids=list(range(NCORES)))
    out = np.stack(
        [np.asarray(res.results[i]["out"]).astype(np.float32) for i in range(NCORES)],
        axis=0,
    )
    return out.reshape(B, C, T, H, W)


def _install_ntff_hook():
    """The image's antenv lacks axon_hooks; synthesize it so
    run_bass_kernel_spmd(trace=True) can capture NTFF profiles."""
    import types

    try:
        from antenv.axon_hooks import get_axon_ntff_profile_hook  # noqa: F401

        return True
    except ImportError:
        pass
    try:
        import antenv

        mod = types.ModuleType("antenv.axon_hooks")
        _state = {"hook": None}

        def set_axon_ntff_profile_hook(h):
            _state["hook"] = h

        def get_axon_ntff_profile_hook():
            return _state["hook"]

        mod.set_axon_ntff_profile_hook = set_axon_ntff_profile_hook
        mod.get_axon_ntff_profile_hook = get_axon_ntff_profile_hook
        sys.modules["antenv.axon_hooks"] = mod
        antenv.axon_hooks = mod

        sys.path.insert(0, "/root/.axon_site")
        from trn_agent_boot.trn_boot import _ntff_profile_via_ctypes

        hook = _ntff_profile_via_ctypes("/opt/axon/libaxon_pjrt.so")
        if hook is None:
            return False
        set_axon_ntff_profile_hook(hook)
        return True
    except Exception as e:  # pragma: no cover
        print("ntff hook install failed:", e)
        return False


def profile_once(inputs):
    """Run with NTFF tracing; returns max per-core exec_time_ns."""
    _install_ntff_hook()
    x = np.asarray(inputs["x"])
    nc = _get_nc()
    xs = np.ascontiguousarray(x).reshape(NCORES, S, C, P, F)
    g = np.asarray(inputs["gamma"], dtype=np.float32).reshape(1, 1)
    cns = const_inputs()
    in_maps = [{"x": xs[i], "gamma": g, **cns} for i in range(NCORES)]
    res = run_bass_kernel_spmd(
        nc, in_maps, core_ids=list(range(NCORES)), trace=True
    )
    print("profile_json:", res.profile_json)
    print("exec_time_ns:", res.exec_time_ns, "mean:", res.mean_exec_time_ns)
    return res.exec_time_ns


if __name__ == "__main__":
    x = np.random.randn(B, C, T, H, W).astype(np.float32)
    gamma = np.zeros((1,), np.float32)
    y = kernel(x, gamma)
    print("ok", y.shape, float(np.abs(y - x).max()))


# revision 15
# speedup vs baseline: 7.0318x; 1.0204x over previous
"""CAM (channel attention module) Trainium2 kernel.

Reference computation (per sample b):
    xf = x[b].reshape(C, N)
    energy = xf @ xf.T                      # [C, C]
    att = softmax(max_row(energy) - energy) # row-wise == softmax(-energy)
    out = gamma * (att @ xf) + xf

Full shapes: x [128, 3, 16, 112, 112] f32, gamma [1] f32.
Data-parallel over batch: 16 samples per core on 8 NeuronCores.

v5 design (per core, 16 samples, streaming):
 - input DMA casts f32->bf16 in the SWDGE (gpsimd) path; SBUF holds bf16.
 - gram on the (otherwise idle) TensorE: for each of the 6 channel pairs,
   13 accumulating [128, <=128]^T @ [128, <=128] matmuls into a
   [128, 6, 128] PSUM tile (walrus requires single-free-dim matmul APs).
   Energies = PSUM diagonals, extracted by 6 DVE STT+accum ops against an
   identity mask, partition-reduced by a ones matmul + W2 gather (tiny).
 - apply (out_c = sum_d mb[c,d] x_d, mb = I + gamma*att) as 9 bf16
   tensor_scalar muls (split DVE/ScalarE; stock TS has 2x/4x uops) and
   2 channel-fused [P, 3F] bf16 tensor_tensor adds on DVE (2x mode).
   scalar_tensor_tensor is avoided for big ops: it only has a 1x uop.
 - output written bf16 (tolerance 2e-2; bf16 round-trip ~2e-3), host
   upconverts to f32.
 - no gpsimd compute (GpSimd and DVE's 2nd port share an exclusive SBUF
   port pair; mixing them serializes both engines).
"""

import sys

sys.path.insert(0, "/opt/trn_rl_repo")

import numpy as np

import concourse.bass as bass
import concourse.tile as tile
from concourse import mybir
from concourse.bass_utils import run_bass_kernel_spmd

B, C, T, H, W = 128, 3, 16, 112, 112
N = T * H * W                 # 200704
P = 128
F = N // P                    # 1568
NCORES = 8
S = B // NCORES               # 16 samples per core

GCH = 64                      # f-chunk per gram matmul
PAIRS6 = [(0, 0), (1, 1), (2, 2), (0, 1), (0, 2), (1, 2)]

FP32 = mybir.dt.float32
BF16 = mybir.dt.bfloat16
AX = mybir.AxisListType
ALU = mybir.AluOpType
ACT = mybir.ActivationFunctionType

# --- tuning knobs -----------------------------------------------------------
CFG = dict(
    swdge_in=True,   # cast f32->bf16 inside the input DMA (gpsimd SWDGE)
    ahead=6,         # gram/extract run this many samples ahead of apply
    in_bufs=9,       # lookahead = in_bufs - 1 samples
    out_bufs=3,
    gram_bufs=2,     # PSUM double-buffering for gram tiles
    group=4,         # samples per batched softmax chain
)
# apply-mul split: ScalarE gets these (d, c) pairs, DVE the rest
MULS_SCALAR = [(0, 0), (0, 1), (0, 2), (1, 0), (1, 1)]
MULS_DVE = [(1, 2), (2, 0), (2, 1), (2, 2)]


def _bcast(ap, n, pos):
    """Insert a 0-stride dim of extent n at position pos of the ap list."""
    new = list(ap.ap)
    new.insert(pos, [0, n])
    return bass.AP(tensor=ap.tensor, offset=ap.offset, ap=new)


def split_multi_waits(nc):
    """This container's walrus accepts only one sync-wait per instruction.
    Hoist extra waits onto single-wait NOPs on the same (in-order) queue."""
    n_split = 0
    for bb in nc.main_func.blocks:
        insts = list(bb.instructions)
        new = []
        for inst in insts:
            si = inst.sync_info
            waits = list(si.on_wait) if si is not None else []
            if len(waits) > 1:
                for i, w in enumerate(waits[:-1]):
                    nop = mybir.InstNoOp(
                        name=f"{inst.name}-wsplit{i}",
                        opcode="NoOp",
                        engine=inst.engine,
                        text_hint="wait_split",
                        bass_nofuse=True,
                        sync_info=mybir.SyncInfo(on_wait=[w], on_update=[]),
                    )
                    new.append(nop)
                    n_split += 1
                inst.sync_info = mybir.SyncInfo(
                    on_wait=[waits[-1]], on_update=list(si.on_update)
                )
            new.append(inst)
        if len(new) != len(insts):
            try:
                bb.instructions = new
            except Exception:
                del bb.instructions[:]
                bb.instructions.extend(new)
    return n_split


def build_kernel(cfg=CFG, s_per_core=S, n_free=F, split_waits=True):
    """Emit the per-core Tile program. DRAM views: [S, C, P, F]."""
    from contextlib import ExitStack

    nc = bass.Bass("TRN2", target_bir_lowering=False, debug=False)
    f = n_free
    # gram chunking: 12 full 128-wide chunks + one 32-wide remainder
    chunks = []
    pos = 0
    while pos < f:
        w = min(GCH, f - pos)
        chunks.append((pos, w))
        pos += w

    x_d = nc.dram_tensor("x", [s_per_core, C, P, f], FP32, kind="ExternalInput")
    g_d = nc.dram_tensor("gamma", [1, 1], FP32, kind="ExternalInput")
    i9_d = nc.dram_tensor("i9c", [1, 9], FP32, kind="ExternalInput")
    w2_d = nc.dram_tensor("w2c", [6, 9], FP32, kind="ExternalInput")
    dg_d = nc.dram_tensor("diagm", [P, P], FP32, kind="ExternalInput")
    w2g_d = nc.dram_tensor(
        "w2g", [6 * cfg["group"], 9 * cfg["group"]], FP32, kind="ExternalInput"
    )
    i9g_d = nc.dram_tensor("i9g", [1, 9 * cfg["group"]], FP32, kind="ExternalInput")
    o_d = nc.dram_tensor("out", [s_per_core, C, P, f], BF16, kind="ExternalOutput")

    with tile.TileContext(nc) as tc, ExitStack() as ctx:
        consts = ctx.enter_context(tc.tile_pool(name="consts", bufs=1))
        in_pool = ctx.enter_context(tc.tile_pool(name="in", bufs=cfg["in_bufs"]))
        out_pool = ctx.enter_context(tc.tile_pool(name="outp", bufs=cfg["out_bufs"]))
        u_pool = ctx.enter_context(tc.tile_pool(name="u", bufs=2))
        small = ctx.enter_context(tc.tile_pool(name="small", bufs=4))
        psum = ctx.enter_context(tc.tile_pool(name="psum", bufs=1, space="PSUM"))
        gpsum = ctx.enter_context(
            tc.tile_pool(name="gpsum", bufs=cfg["gram_bufs"], space="PSUM")
        )

        # ---- constants ----
        ones_k = consts.tile([P, 1], FP32)          # partition-reduce rhs
        nc.vector.memset(ones_k, 1.0)
        ones_b = consts.tile([1, P], FP32)          # K=1 broadcast lhsT
        nc.vector.memset(ones_b, 1.0)
        i9 = consts.tile([1, 9], FP32)              # flat 3x3 identity
        nc.sync.dma_start(out=i9, in_=i9_d.ap())
        w2 = consts.tile([6, 9], FP32)              # pair -> (c,d) gather
        nc.sync.dma_start(out=w2, in_=w2_d.ap())
        w2g = consts.tile([6 * CFG["group"], 9 * CFG["group"]], FP32)
        nc.sync.dma_start(out=w2g, in_=w2g_d.ap())
        i9g = consts.tile([1, 9 * CFG["group"]], FP32)
        nc.sync.dma_start(out=i9g, in_=i9g_d.ap())
        diagm = consts.tile([P, P], FP32)           # 128x128 identity mask
        nc.sync.dma_start(out=diagm, in_=dg_d.ap())
        gamma_sb = consts.tile([1, 1], FP32)
        nc.sync.dma_start(out=gamma_sb, in_=g_d.ap())

        xin_tiles = {}
        gram_tiles = {}
        mb_tiles = {}

        def emit_load(si):
            xin = in_pool.tile([P, C, f], BF16, tag="xin")
            src = x_d.ap()[si].rearrange("c p f -> p c f")
            if cfg["swdge_in"]:
                nc.gpsimd.dma_start(out=xin, in_=src)
            else:
                nc.sync.dma_start(out=xin, in_=src)
            xin_tiles[si] = xin

        def emit_gram(si):
            """TensorE: M[j][f,f'] = sum_n-chunks x_c[:,f] . x_d[:,f'] per pair."""
            xin = xin_tiles[si]
            m_ps = gpsum.tile([GCH, 6, GCH], FP32, tag="gram")
            for k, (p0, w) in enumerate(chunks):
                for j, (a, b) in enumerate(PAIRS6):
                    nc.tensor.matmul(
                        out=m_ps[:w, j, :w],
                        lhsT=xin[:, a, p0 : p0 + w],
                        rhs=xin[:, b, p0 : p0 + w],
                        start=(k == 0),
                        stop=(k == len(chunks) - 1),
                        skip_group_check=True,
                    )
            gram_tiles[si] = m_ps

        def emit_extract(si, dsum, slot):
            """diag sums of the 6 PSUM pair blocks -> dsum[:, 6*slot:6*slot+6]."""
            m_ps = gram_tiles[si]
            scr = small.tile([GCH, GCH], BF16, tag="scr")
            for j in range(6):
                nc.vector.scalar_tensor_tensor(
                    out=scr,
                    in0=m_ps[:GCH, j, :],
                    scalar=1.0,
                    in1=diagm[:GCH, :GCH],
                    op0=ALU.mult,
                    op1=ALU.mult,
                    accum_out=dsum[:, 6 * slot + j : 6 * slot + j + 1],
                )
            del gram_tiles[si]

        GRP = cfg["group"]

        def emit_chain_group(g, dsum):
            """batched softmax chain for GRP samples -> mb [P, 9*GRP]."""
            n9 = 9 * GRP
            p1t_ps = psum.tile([6 * GRP, 1], FP32, tag="p1t")
            nc.tensor.matmul(out=p1t_ps, lhsT=dsum, rhs=ones_k[:GCH])
            p1t = small.tile([6 * GRP, 1], FP32, tag="p1t_sb")
            nc.scalar.copy(p1t, p1t_ps)
            e_ps = psum.tile([1, n9], FP32, tag="e")
            nc.tensor.matmul(out=e_ps, lhsT=p1t, rhs=w2g)
            e_sb = small.tile([1, n9], FP32, tag="e_sb")
            nc.scalar.copy(e_sb, e_ps)
            e3 = e_sb.rearrange("p (sc d) -> p sc d", d=3)
            rmin = small.tile([1, 3 * GRP], FP32, tag="rmin")
            nc.vector.tensor_reduce(out=rmin, in_=e3, axis=AX.X, op=ALU.min)
            z = small.tile([1, n9], FP32, tag="z")
            nc.vector.scalar_tensor_tensor(
                out=z.rearrange("p (sc d) -> p sc d", d=3),
                in0=e3,
                scalar=-1.0,
                in1=_bcast(rmin, 3, 2),
                op0=ALU.mult,
                op1=ALU.add,
            )
            ex = small.tile([1, n9], FP32, tag="ex")
            nc.scalar.activation(out=ex, in_=z, func=ACT.Exp)
            ex3 = ex.rearrange("p (sc d) -> p sc d", d=3)
            sm = small.tile([1, 3 * GRP], FP32, tag="sm")
            nc.vector.tensor_reduce(out=sm, in_=ex3, axis=AX.X, op=ALU.add)
            lnsm = small.tile([1, 3 * GRP], FP32, tag="lnsm")
            nc.scalar.activation(out=lnsm, in_=sm, func=ACT.Ln)
            w = small.tile([1, n9], FP32, tag="w")
            nc.vector.scalar_tensor_tensor(
                out=w.rearrange("p (sc d) -> p sc d", d=3),
                in0=z.rearrange("p (sc d) -> p sc d", d=3),
                scalar=1.0,
                in1=_bcast(lnsm, 3, 2),
                op0=ALU.mult,
                op1=ALU.subtract,
            )
            att = small.tile([1, n9], FP32, tag="att")
            nc.scalar.activation(out=att, in_=w, func=ACT.Exp)
            mflat = small.tile([1, n9], FP32, tag="mflat")
            nc.vector.scalar_tensor_tensor(
                out=mflat, in0=att, scalar=gamma_sb, in1=i9g, op0=ALU.mult, op1=ALU.add
            )
            mb_ps = psum.tile([P, n9], FP32, tag="mb")
            nc.tensor.matmul(out=mb_ps, lhsT=ones_b, rhs=mflat)
            mb = small.tile([P, n9], FP32, tag="mb_sb")
            nc.scalar.copy(mb, mb_ps)
            mb_tiles[g] = mb

        u_tiles = {}

        def emit_muls(si):
            """U_d[:, c, :] = x_d * mb[c,d] (9 scalar muls, split engines)."""
            xin = xin_tiles[si]
            mb = mb_tiles[si // GRP]
            off = 9 * (si % GRP)
            u0 = u_pool.tile([P, C, f], BF16, tag="u0")
            u1 = u_pool.tile([P, C, f], BF16, tag="u1")
            u2 = u_pool.tile([P, C, f], BF16, tag="u2")
            us = [u0, u1, u2]
            for d, c in MULS_SCALAR:
                nc.scalar.mul(
                    us[d][:, c, :], xin[:, d, :],
                    mb[:, off + 3 * c + d : off + 3 * c + d + 1],
                )
            for d, c in MULS_DVE:
                nc.vector.tensor_scalar_mul(
                    us[d][:, c, :], xin[:, d, :],
                    mb[:, off + 3 * c + d : off + 3 * c + d + 1],
                )
            u_tiles[si] = us
            del xin_tiles[si]

        def emit_apply_tt(si):
            """out = U0 + U1 + U2 (c-fused TT adds) + store."""
            u0, u1, u2 = u_tiles[si]
            tsum = u_pool.tile([P, C, f], BF16, tag="tsum")
            nc.vector.tensor_tensor(out=tsum, in0=u0, in1=u1, op=ALU.add)
            outt = out_pool.tile([P, C, f], BF16, tag="outt")
            nc.vector.tensor_tensor(out=outt, in0=tsum, in1=u2, op=ALU.add)
            nc.sync.dma_start(out=o_d.ap()[si].rearrange("c p f -> p c f"), in_=outt)
            del u_tiles[si]

        # ---- software pipeline ----
        # U muls run one sample ahead of the TT adds; gram/extract run
        # AHEAD samples ahead so the batched chain (one per GRP samples)
        # lands ~2 iterations before its group's muls need mb.
        AHEAD = cfg["ahead"]
        lookahead = cfg["in_bufs"] - 1
        dsums = {}

        def group_dsum(g):
            if g not in dsums:
                dsums[g] = small.tile([GCH, 6 * GRP], FP32, tag="dsum", name=f"ds{g}")
            return dsums[g]

        for si in range(min(lookahead, s_per_core)):
            emit_load(si)
        for si in range(min(AHEAD, s_per_core)):
            emit_gram(si)
            emit_extract(si, group_dsum(si // GRP), si % GRP)
            if si % GRP == GRP - 1:
                emit_chain_group(si // GRP, dsums[si // GRP])
        emit_muls(0)
        for s in range(s_per_core):
            if s + lookahead < s_per_core:
                emit_load(s + lookahead)
            s2 = s + AHEAD
            if s2 < s_per_core:
                emit_gram(s2)
            emit_apply_tt(s)
            if s2 < s_per_core:
                emit_extract(s2, group_dsum(s2 // GRP), s2 % GRP)
            if s + 1 < s_per_core:
                emit_muls(s + 1)
            if s2 < s_per_core and s2 % GRP == GRP - 1:
                emit_chain_group(s2 // GRP, dsums[s2 // GRP])

    if split_waits:
        split_multi_waits(nc)
    return nc


def const_inputs():
    i9 = np.eye(3, dtype=np.float32).reshape(1, 9)
    w2 = np.zeros((6, 9), np.float32)
    for j, (a, b) in enumerate(PAIRS6):
        w2[j, 3 * a + b] = 1.0
        w2[j, 3 * b + a] = 1.0
    diagm = np.eye(P, dtype=np.float32)
    g = CFG["group"]
    w2g = np.kron(np.eye(g, dtype=np.float32), w2)
    i9g = np.tile(i9, (1, g))
    return {"i9c": i9, "w2c": w2, "diagm": diagm, "w2g": w2g, "i9g": i9g}


_NC_CACHE = {}


def _get_nc():
    key = "full"
    if key not in _NC_CACHE:
        _NC_CACHE[key] = build_kernel()
    return _NC_CACHE[key]


def kernel(x: np.ndarray, gamma: np.ndarray) -> np.ndarray:
    assert x.shape == (B, C, T, H, W) and x.dtype == np.float32
    nc = _get_nc()
    xs = np.ascontiguousarray(x).reshape(NCORES, S, C, P, F)
    g = np.asarray(gamma, dtype=np.float32).reshape(1, 1)
    cns = const_inputs()
    in_maps = [{"x": xs[i], "gamma": g, **cns} for i in range(NCORES)]
    res = run_bass_kernel_spmd(nc, in_maps, core_ids=list(range(NCORES)))
    out = np.stack(
        [np.asarray(res.results[i]["out"]).astype(np.float32) for i in range(NCORES)],
        axis=0,
    )
    return out.reshape(B, C, T, H, W)


def _install_ntff_hook():
    """The image's antenv lacks axon_hooks; synthesize it so
    run_bass_kernel_spmd(trace=True) can capture NTFF profiles."""
    import types

    try:
        from antenv.axon_hooks import get_axon_ntff_profile_hook  # noqa: F401

        return True
    except ImportError:
        pass
    try:
        import antenv

        mod = types.ModuleType("antenv.axon_hooks")
        _state = {"hook": None}

        def set_axon_ntff_profile_hook(h):
            _state["hook"] = h

        def get_axon_ntff_profile_hook():
            return _state["hook"]

        mod.set_axon_ntff_profile_hook = set_axon_ntff_profile_hook
        mod.get_axon_ntff_profile_hook = get_axon_ntff_profile_hook
        sys.modules["antenv.axon_hooks"] = mod
        antenv.axon_hooks = mod

        sys.path.insert(0, "/root/.axon_site")
        from trn_agent_boot.trn_boot import _ntff_profile_via_ctypes

        hook = _ntff_profile_via_ctypes("/opt/axon/libaxon_pjrt.so")
        if hook is None:
            return False
        set_axon_ntff_profile_hook(hook)
        return True
    except Exception as e:  # pragma: no cover
        print("ntff hook install failed:", e)
        return False


def profile_once(inputs):
    """Run with NTFF tracing; returns max per-core exec_time_ns."""
    _install_ntff_hook()
    x = np.asarray(inputs["x"])
    nc = _get_nc()
    xs = np.ascontiguousarray(x).reshape(NCORES, S, C, P, F)
    g = np.asarray(inputs["gamma"], dtype=np.float32).reshape(1, 1)
    cns = const_inputs()
    in_maps = [{"x": xs[i], "gamma": g, **cns} for i in range(NCORES)]
    res = run_bass_kernel_spmd(
        nc, in_maps, core_ids=list(range(NCORES)), trace=True
    )
    print("profile_json:", res.profile_json)
    print("exec_time_ns:", res.exec_time_ns, "mean:", res.mean_exec_time_ns)
    return res.exec_time_ns


if __name__ == "__main__":
    x = np.random.randn(B, C, T, H, W).astype(np.float32)
    gamma = np.zeros((1,), np.float32)
    y = kernel(x, gamma)
    print("ok", y.shape, float(np.abs(y - x).max()))
